# revision 26
# baseline (speedup 1.0000x reference)
"""GAT layer kernel for Trainium2, 8 NeuronCores.

Reference computation:
    X = node_features @ W            [N, DOUT]
    f0 = X @ v0 ; f1 = X @ v1       [N, 1]
    vals = sigmoid(f0 + f1.T) - 0.5
    alphas = softmax(where(graph != 0, vals, -inf), axis=1), masked to 0
    out = elu(alphas @ X)

Design notes:
  * softmax(sigmoid(z) - 0.5) == softmax(sigmoid(z)) (row-constant shift),
    so the softmax weights are w = exp(sigmoid(z)) up to a global scale.
  * w is evaluated in ONE activation pass through a patched ACT spline
    table: the `exp` buckets for |x| < 32 are refit to g(x)=exp(sigmoid(x))
    (bucket centers and ctrl tables unchanged, only cubic coefficients),
    while [32,128) keeps true exp so the elu epilogue can compute
    exp(u) = exp(u + 80) * e^-80 out of the un-patched region.
  * The adjacency mask ships as bf16 {0,1}, streamed by plain SWDGE DMA
    (tiled layout, 8KB contiguous runs per partition) on the otherwise-idle
    GpSimd queue, and applied multiplicatively with a single DVE
    tensor_tensor per group (bf16 2x packed mode). w >= 1 for unmasked
    entries so multiply-by-{0,1} is an exact mask.
  * softmax ratio: out_row = (sum_j p_ij X_j) / (sum_j p_ij); the row-sum
    comes free as a ones-column in the matmul rhs.
  * Two-phase schedule: ALL X~ chunks stream first (interleaved in pairs
    across the two PSUM scratch banks so LDWEIGHTS overlaps the previous
    matmul), freeing those banks before any attention matmul issues; the
    attention then runs 8 row-blocks wide with no tail sweep, overlapping
    the ACT weight pass which paces the second phase.
  * Row-sharding: each core owns N/8 output rows; each core recomputes
    X~ = nf @ [W | W@v0 | W@v1] for ALL rows from a replicated bf16 nf^T
    (collectives measured ~90us of barrier+skew on this fabric).
  * Per-core the j-axis is rotated host-side so the core's own rows come
    first: f0 (needed by every attention chunk) is computed from the
    first two streamed nf groups, with a K=1 matmul broadcasting the f0
    row to all 128 partitions. Softmax sums are order-invariant.
"""

import json
import os
import shutil
import tempfile

import numpy as np

# ----------------------------------------------------------------------------
# ACT table patch: refit exp's buckets to g(x)=exp(sigmoid(x)) for |x|<32,
# zero for x in (-128,-32], true exp kept for x in [32,128). Entry format
# (8 x fp32): [d0, d1, d2, d3, x0, 0, 0, 0], y = d0+d1 t+d2 t^2+d3 t^3,
# t = x - x0. Centers/ctrl/profile structure unchanged.
# ----------------------------------------------------------------------------

_ACT_SET = "exp_and_others"


def _g_target(x):
    return np.exp(1.0 / (1.0 + np.exp(-x)))


def _fit_cubic(f, lo, hi):
    x0 = 0.5 * (lo + hi)
    xs = x0 + 0.5 * (hi - lo) * np.cos(np.linspace(0, np.pi, 33))
    t = (xs - x0).astype(np.float64)
    A = np.stack([np.ones_like(t), t, t * t, t ** 3], axis=1)
    coef, *_ = np.linalg.lstsq(A, f(xs.astype(np.float64)), rcond=None)
    return coef, x0


def _patched_act_tables():
    """Write a patched copy of the pwp table dir; return act_info.json path."""
    from neuronxcc.driver.Job import Job

    src = os.path.join(Job.getPackageDir(), "pwp", "pwp_bin_trainium")
    dst = os.path.join(tempfile.gettempdir(), "gat_actpatch_v1")
    marker = os.path.join(dst, ".done")
    info = os.path.join(dst, "act_info.json")
    if os.path.exists(marker):
        return info
    shutil.rmtree(dst, ignore_errors=True)
    os.makedirs(dst)
    for f in os.listdir(src):
        shutil.copy(os.path.join(src, f), os.path.join(dst, f))
        os.chmod(os.path.join(dst, f), 0o644)

    bkt = np.fromfile(os.path.join(dst, f"{_ACT_SET}_bkt.bin"), dtype=np.float32)
    bkt = bkt.reshape(-1, 8).copy()
    prof = json.load(open(os.path.join(dst, f"{_ACT_SET}.json")))

    groups = {}
    for i in range(781):
        if i in (777, 778, 779, 780):
            continue
        x0 = float(bkt[i, 4])
        if x0 == 0.0:
            continue
        e = int(np.floor(np.log2(abs(x0))))
        groups.setdefault((np.sign(x0), e), []).append(i)
    for (sgn, e), idxs in groups.items():
        idxs.sort(key=lambda i: bkt[i, 4])
        centers = bkt[idxs, 4].astype(np.float64)
        w = float(np.min(np.diff(centers))) if len(idxs) > 1 else float(2.0 ** e)
        for i in idxs:
            x0 = float(bkt[i, 4])
            if x0 > 0 and e >= 5:
                continue          # keep true exp on [32, 128): elu epilogue
            if x0 < 0 and e >= 5:
                bkt[i, 0:4] = 0.0  # (-128, -32]: zero (unreachable margin)
                continue
            coef, _ = _fit_cubic(_g_target, x0 - w / 2, x0 + w / 2)
            bkt[i, 0:4] = coef.astype(np.float32)
    g0 = float(np.exp(0.5))
    for i in (777, 778):           # |x| < 2^-19 small-signal buckets
        bkt[i, 0:4] = [g0, 0.25 * g0, 0.5 * g0 * 0.25 ** 2, 0.0]
        bkt[i, 4] = 0.0
    for ent in prof["profile_meta_data"]:
        if ent["func_name"].startswith("exp_"):
            ent["fzero_result"] = int(np.float32(g0).view(np.uint32))

    bkt.astype(np.float32).tofile(os.path.join(dst, f"{_ACT_SET}_bkt.bin"))
    with open(os.path.join(dst, f"{_ACT_SET}.json"), "w") as fh:
        json.dump(prof, fh)
    open(marker, "w").close()
    return info


os.environ["BASS_ACT_ROOT_JSON_PATH"] = _patched_act_tables()

import concourse.bass as bass
import concourse.mybir as mybir
import concourse.tile as tile
from concourse.bass_utils import run_bass_kernel_spmd

# ----------------------------------------------------------------------------
# Workaround for "Too many sync wait commands": this walrus build accepts only
# ONE sync-wait per instruction. Post-pass: hoist surplus waits onto
# single-wait NOPs on the same engine, inserted immediately before the
# instruction (identical blocking semantics, per-engine order preserved).
# ----------------------------------------------------------------------------


def _split_multi_waits(nc):
    import bass_rust

    eng = {
        mybir.EngineType.PE: nc.tensor,
        mybir.EngineType.DVE: nc.vector,
        mybir.EngineType.Activation: nc.scalar,
        mybir.EngineType.Pool: nc.gpsimd,
        mybir.EngineType.SP: nc.sync,
    }
    for f in nc.m.functions:
        for blk in f.blocks:
            fixups = []  # (index, inst, waits)
            for idx, inst in enumerate(blk.instructions):
                si = inst.sync_info
                waits = list(si.on_wait) if si is not None and si.on_wait else []
                if len(waits) > 1 and inst.engine in eng:
                    fixups.append((idx, inst, waits))
            if not fixups:
                continue
            nops_by_idx = {}
            created = set()
            for idx, inst, waits in fixups:
                inst.sync_info.on_wait = [waits[-1]]
                nops = []
                for w in waits[:-1]:
                    nop = eng[inst.engine].nop(nofuse=True, hint="wait_split").ins
                    nop.sync_info = bass_rust.SyncInfo(on_wait=[w], on_update=[])
                    nops.append(nop)
                    created.add(id(nop))
                nops_by_idx[idx] = nops
            for b2 in f.blocks:
                b2.instructions[:] = [
                    i for i in b2.instructions if id(i) not in created
                ]
            new = []
            for idx, inst in enumerate(blk.instructions):
                new.extend(nops_by_idx.get(idx, ()))
                new.append(inst)
            blk.instructions[:] = new


# ----------------------------------------------------------------------------

F32 = mybir.dt.float32
BF16 = mybir.dt.bfloat16
AF = mybir.ActivationFunctionType
ALU = mybir.AluOpType

N, D_IN, D_OUT = 8192, 512, 256
M_CORES = 8
P = 128
EXP_SHIFT = 80.0  # elu exp computed as exp(u+80)*e^-80 (un-patched region)


def build_gat(n=N, d_in=D_IN, d_out=D_OUT, m_cores=M_CORES, grp=4, debug=False):
    """Per-core SPMD program. Inputs per core (j-axis rotated so own rows
    come first):
      nfT   [d_in, n]  bf16  -- node_features.T, columns rotated per core
      maskT [P, n_grp*grp*R] bf16 -- {1 unmasked, 0 masked}, tiled layout
      wext  [d_in, d_out+2] bf16 -- [W | W@v0 | W@v1]
    Output: out [R, d_out] f32 (this core's rows)."""
    R = n // m_cores
    NJ = n // P                  # 64 j-chunks
    IB = R // P                  # 8 output row-blocks
    DK = d_in // P               # 4 contraction chunks
    DEXT = d_out + 2
    DW = d_out + 1               # attention rhs: X | ones
    XW = d_out + 2               # xsb row: X | ones | f1
    n_grp = NJ // grp            # 16 groups
    JG = grp * P                 # 512 j per group

    nc = bass.Bass(num_devices=m_cores)
    # All inputs ship partition-major-tiled so each DMA has >=4KB contiguous
    # runs per partition (1KB-run APs measured ~75GB/s vs ~350 line rate):
    #   nfT[dp, g, kc, j]   = nf.T[kc*128+dp, g*512+j]
    #   maskT[dp, g, bb, i] = mask01.T[g*512+bb*128+dp, i]
    #   wext[dp, kc, c]     = [W|Wv0|Wv1][kc*128+dp, c]
    maskT = nc.declare_dram_parameter("maskT", [P, n_grp * grp * R], BF16, isOutput=False)
    nfT = nc.declare_dram_parameter("nfT", [P, n_grp * DK * JG], BF16, isOutput=False)
    wext = nc.declare_dram_parameter("wext", [P, DK * DEXT], BF16, isOutput=False)
    outp = nc.declare_dram_parameter("out", [R, d_out], F32, isOutput=True)

    with tile.TileContext(nc) as tc:
        with tc.tile_pool(name="persist", bufs=1) as persist, \
             tc.tile_pool(name="nfc", bufs=2) as nf_pool, \
             tc.tile_pool(name="mk", bufs=2) as mk_pool, \
             tc.tile_pool(name="pg", bufs=16) as p_pool, \
             tc.tile_pool(name="epi", bufs=2) as epi:

            wextb = persist.tile([P, DK, DEXT], BF16)
            nc.sync.dma_start(
                out=wextb,
                in_=bass.AP(wext, 0, [[DK * DEXT, P], [DEXT, DK], [1, DEXT]]),
            )
            # prewarm the (patched) exp table during the preamble
            warm = persist.tile([P, 1], F32)
            nc.gpsimd.memset(warm, 40.0)   # true-exp region, harmless
            nc.scalar.activation(out=warm, in_=warm, func=AF.Exp)

            # xsb row layout: [X (256) | ones (256) | f1 (257)]; the ones
            # column is re-memset per group after the 258-wide PSUM cast
            # (which drops f0-junk there); attention rhs reads cols 0:257.
            xsb = persist.tile([P, NJ, XW], BF16)
            f0rep = persist.tile([P, R], F32)
            f0flat = persist.tile([1, R], F32)
            ones1 = persist.tile([1, P], F32)
            nc.vector.memset(ones1, 1.0)
            b80 = persist.tile([P, 1], F32)
            nc.vector.memset(b80, EXP_SHIFT)

            pg_tiles = []
            mk_tiles = {}
            with tc.tile_pool(name="ps1", bufs=1, space="PSUM") as ps1:
                # 8 PSUM banks, fully subscribed:
                #   5 dedicated attention accumulators (row-blocks 0-4)
                #   1 shared bank, serially: f0 row psums -> f0 broadcast
                #     halves -> 6th accumulator (row-block 5)
                #   2 X~ tile slots; after the last X~ tile their slots
                #     become accumulators for row-blocks 6-7 (tail).
                # Row-blocks 0-5 accumulate DURING the X~/ACT streaming
                # phase (interleaved per group, 2 groups behind so PE never
                # waits on the mask+mult latency); 6-7 run as a short tail.
                acc = [
                    ps1.tile([P, DW], F32, tag=f"acc{ib}", name=f"acc{ib}")
                    for ib in range(5)
                ]
                NIB1 = 6

                def sh1(shape, name):
                    return ps1.tile(shape, F32, tag="sh1", name=name, bufs=1)

                def emit_attn(g, ibs):
                    for bb in range(grp):
                        chunk = g * grp + bb
                        for ib in ibs:
                            nc.tensor.matmul(
                                out=acc[ib],
                                lhsT=pg_tiles[g][:, bb, ib * P:(ib + 1) * P],
                                rhs=xsb[:, chunk, 0:DW],
                                start=(chunk == 0),
                                stop=(chunk == NJ - 1),
                            )

                def emit_weights(g):
                    """ACT pass + multiplicative {0,1} mask for group g. Must
                    only be emitted once f0rep's write has been emitted (reads
                    emitted before a tile's first write see garbage)."""
                    pg = pg_tiles[g]
                    # one ACT pass: p = g(f0_i + f1_j) = exp(sigmoid(z))
                    for bb in range(grp):
                        jc = g * grp + bb
                        nc.scalar.activation(
                            out=pg[:, bb, :],
                            in_=f0rep,
                            func=AF.Exp,
                            bias=xsb[:, jc, d_out + 1:d_out + 2],
                            scale=1.0,
                        )
                    # mask: fused bf16 multiplies (2x packed DVE mode), one
                    # per chunk so each attention chunk is gated only by its
                    # own multiply; p >= 1 for unmasked entries so
                    # {0,1}-multiply is exact
                    mk = mk_tiles.pop(g)
                    for bb in range(grp):
                        nc.vector.tensor_tensor(
                            out=pg[:, bb, :],
                            in0=pg[:, bb, :],
                            in1=mk[:, bb, :],
                            op=ALU.mult,
                        )

                f0ps = [None, None]

                def emit_group_x(g):
                    """nf + mask DMAs, f0 rows (g<2), X~ matmuls + casts."""
                    nfc = nf_pool.tile([P, DK, JG], BF16, tag="nfc")
                    nc.sync.dma_start(
                        out=nfc,
                        in_=bass.AP(
                            nfT, g * DK * JG,
                            [[n_grp * DK * JG, P], [JG, DK], [1, JG]],
                        ),
                    )
                    # mask tile for this group: SWDGE on the idle GpSimd
                    # queue so its buffer-waits never stall the nf stream
                    mk = mk_pool.tile([P, grp, R], BF16, tag="mk")
                    mk_tiles[g] = mk
                    nc.gpsimd.dma_start(
                        out=mk,
                        in_=bass.AP(
                            maskT, g * grp * R,
                            [[n_grp * grp * R, P], [R, grp], [1, R]],
                        ),
                    )
                    # f0 row for own rows (groups 0-1 under rotation), via
                    # stationary wv0 column (M=1) -> row-form, no transpose
                    if g < 2:
                        f0ps[g] = sh1([1, JG], f"f0ps{g}")
                        for kc in range(DK):
                            nc.tensor.matmul(
                                out=f0ps[g],
                                lhsT=wextb[:, kc, d_out:d_out + 1],
                                rhs=nfc[:, kc, :],
                                start=(kc == 0),
                                stop=(kc == DK - 1),
                            )
                        # ScE copy: keeps the f0 chain off the busy DVE
                        nc.scalar.activation(
                            out=f0flat[:, g * JG:(g + 1) * JG], in_=f0ps[g],
                            func=AF.Copy,
                        )
                    pg = p_pool.tile([P, grp, R], BF16, tag="pg")
                    pg_tiles.append(pg)
                    # X~ for 4 chunks, as 2 interleaved pairs ping-ponging
                    # the two PSUM scratch banks (LDWEIGHTS of one matmul
                    # overlaps the other bank's matmul stream)
                    for bb in (0, 2):
                        xa = ps1.tile([P, DEXT], F32, tag="xps", bufs=2)
                        xb = ps1.tile([P, DEXT], F32, tag="xps", bufs=2)
                        for kc in range(DK):
                            nc.tensor.matmul(
                                out=xa,
                                lhsT=nfc[:, kc, bb * P:(bb + 1) * P],
                                rhs=wextb[:, kc, :],
                                start=(kc == 0),
                                stop=(kc == DK - 1),
                            )
                            nc.tensor.matmul(
                                out=xb,
                                lhsT=nfc[:, kc, (bb + 1) * P:(bb + 2) * P],
                                rhs=wextb[:, kc, :],
                                start=(kc == 0),
                                stop=(kc == DK - 1),
                            )
                        # 258-wide casts: X -> 0:256, f0-junk -> 256 (ones
                        # re-memset below), f1 -> 257 (ACT bias reads it)
                        jc = g * grp + bb
                        nc.vector.tensor_copy(
                            out=xsb[:, jc, 0:DEXT], in_=xa[:, 0:DEXT]
                        )
                        nc.vector.tensor_copy(
                            out=xsb[:, jc + 1, 0:DEXT], in_=xb[:, 0:DEXT]
                        )
                    # restore the ones column for this group's 4 chunks
                    nc.vector.memset(xsb[:, g * grp:(g + 1) * grp, d_out], 1.0)

                for g in range(n_grp):
                    emit_group_x(g)
                    if g == 1:
                        # broadcast f0 row to all partitions via K=1 matmul,
                        # two halves serially through the shared bank
                        for h in range(R // JG):
                            fb = sh1([P, JG], f"f0bc{h}")
                            nc.tensor.matmul(
                                out=fb,
                                lhsT=ones1,
                                rhs=f0flat[:, h * JG:(h + 1) * JG],
                                start=True, stop=True,
                            )
                            nc.scalar.activation(
                                out=f0rep[:, h * JG:(h + 1) * JG], in_=fb,
                                func=AF.Copy,
                            )
                        emit_weights(0)
                        emit_weights(1)
                    elif g >= 2:
                        emit_weights(g)
                    if g >= 2:
                        if g == 2:
                            acc.append(sh1([P, DW], "acc5"))
                        emit_attn(g - 2, range(NIB1))
                emit_attn(n_grp - 2, range(NIB1))
                emit_attn(n_grp - 1, range(NIB1))

                kexp = float(np.exp(-EXP_SHIFT))

                def emit_epilogue(ib):
                    # out_row = elu(num / rowsum); rowsum >= 1 always (every
                    # row has a neighbor and p >= 1), so no guard needed.
                    r = epi.tile([P, 1], F32, tag="r")
                    nc.vector.reciprocal(out=r, in_=acc[ib][:, d_out:DW])
                    u = epi.tile([P, d_out], F32, tag="u")
                    nc.vector.tensor_scalar(
                        out=u, in0=acc[ib][:, 0:d_out], scalar1=r, scalar2=None,
                        op0=ALU.mult,
                    )
                    rp = epi.tile([P, d_out], F32, tag="rp")
                    nc.vector.tensor_scalar(
                        out=rp, in0=u, scalar1=0.0, scalar2=-1.0,
                        op0=ALU.max, op1=ALU.add,
                    )
                    xm = epi.tile([P, d_out], F32, tag="xm")
                    nc.vector.tensor_scalar_min(xm, u, 0.0)
                    en = epi.tile([P, d_out], F32, tag="en")
                    nc.scalar.activation(out=en, in_=xm, func=AF.Exp, bias=b80)
                    res = epi.tile([P, d_out], F32, tag="res")
                    nc.vector.scalar_tensor_tensor(
                        out=res, in0=en, scalar=kexp, in1=rp,
                        op0=ALU.mult, op1=ALU.add,
                    )
                    nc.sync.dma_start(out=outp[ib * P:(ib + 1) * P, :], in_=res)

                # blocks 0-5 epilogue overlaps the 6-7 matmul tail
                for ib in range(NIB1):
                    emit_epilogue(ib)

                # tail: last two row-blocks through the freed X~ slots
                acc.append(ps1.tile([P, DEXT], F32, tag="xps", name="acc6",
                                    bufs=2)[:, 0:DW])
                acc.append(ps1.tile([P, DEXT], F32, tag="xps", name="acc7",
                                    bufs=2)[:, 0:DW])
                for chunk in range(NJ):
                    g, bb = divmod(chunk, grp)
                    for ib in (6, 7):
                        nc.tensor.matmul(
                            out=acc[ib],
                            lhsT=pg_tiles[g][:, bb, ib * P:(ib + 1) * P],
                            rhs=xsb[:, chunk, 0:DW],
                            start=(chunk == 0),
                            stop=(chunk == NJ - 1),
                        )
                emit_epilogue(6)
                emit_epilogue(7)

    _split_multi_waits(nc)
    return nc


_cached = {}

# Dev/test knobs (the grading harness just calls kernel(**inputs)):
_TRACE = False
_TMPDIR = None
_LAST_EXEC_NS = None
_LAST_RESULTS = None


def _get_program(n, d_in, d_out, m_cores):
    key = (n, d_in, d_out, m_cores)
    if key not in _cached:
        _cached[key] = build_gat(n, d_in, d_out, m_cores)
    return _cached[key]


def kernel(node_features, graph, W, v0, v1):
    import ml_dtypes

    node_features = np.asarray(node_features, dtype=np.float32)
    graph = np.asarray(graph)
    W = np.asarray(W, dtype=np.float32)
    v0 = np.asarray(v0, dtype=np.float32)
    v1 = np.asarray(v1, dtype=np.float32)

    n, d_in = node_features.shape
    d_out = W.shape[1]
    m = M_CORES
    R = n // m

    nc = _get_program(n, d_in, d_out, m)

    DK, n_grp, JG, grp = d_in // P, n // P // 4, 4 * P, 4
    wext = np.concatenate([W, W @ v0, W @ v1], axis=1).astype(ml_dtypes.bfloat16)
    wext_t = np.ascontiguousarray(
        wext.reshape(DK, P, d_out + 2).transpose(1, 0, 2).reshape(P, -1)
    )
    nfT = node_features.T.astype(ml_dtypes.bfloat16)
    mask01 = (graph != 0).astype(ml_dtypes.bfloat16)

    in_maps = []
    for c in range(m):
        rows = slice(c * R, (c + 1) * R)
        roll = -c * R
        nf_c = np.roll(nfT, roll, axis=1)          # [d_in, n], own cols first
        nf_t = np.ascontiguousarray(
            nf_c.reshape(DK, P, n_grp, JG).transpose(1, 2, 0, 3).reshape(P, -1)
        )
        m_c = np.roll(mask01[rows].T, roll, axis=0)    # [n, R]
        m_t = np.ascontiguousarray(
            m_c.reshape(n_grp, grp, P, R).transpose(2, 0, 1, 3).reshape(P, -1)
        )
        in_maps.append({"nfT": nf_t, "maskT": m_t, "wext": wext_t})
    global _LAST_EXEC_NS, _LAST_RESULTS
    res = run_bass_kernel_spmd(
        nc, in_maps, list(range(m)), trace=_TRACE, tmpdir=_TMPDIR
    )
    _LAST_EXEC_NS = res.exec_time_ns
    _LAST_RESULTS = res
    return np.concatenate([res.results[c]["out"] for c in range(m)], axis=0)


# revision 27
# speedup vs baseline: 1.0740x; 1.0740x over previous
"""GAT layer kernel for Trainium2, 8 NeuronCores.

Reference computation:
    X = node_features @ W            [N, DOUT]
    f0 = X @ v0 ; f1 = X @ v1       [N, 1]
    vals = sigmoid(f0 + f1.T) - 0.5
    alphas = softmax(where(graph != 0, vals, -inf), axis=1), masked to 0
    out = elu(alphas @ X)

Design notes:
  * softmax(sigmoid(z) - 0.5) == softmax(sigmoid(z)) (row-constant shift),
    so the softmax weights are w = exp(sigmoid(z)) up to a global scale.
  * w is evaluated in ONE activation pass through a patched ACT spline
    table: the `exp` buckets for |x| < 32 are refit to g(x)=exp(sigmoid(x))
    (bucket centers and ctrl tables unchanged, only cubic coefficients),
    while [32,128) keeps true exp so the elu epilogue can compute
    exp(u) = exp(u + 80) * e^-80 out of the un-patched region.
  * The adjacency mask ships as bf16 {0,1}, streamed by plain SWDGE DMA
    (tiled layout, 8KB contiguous runs per partition) on the otherwise-idle
    GpSimd queue, and applied multiplicatively with a single DVE
    tensor_tensor per group (bf16 2x packed mode). w >= 1 for unmasked
    entries so multiply-by-{0,1} is an exact mask.
  * softmax ratio: out_row = (sum_j p_ij X_j) / (sum_j p_ij); the row-sum
    comes free as a ones-column in the matmul rhs.
  * Two-phase schedule: ALL X~ chunks stream first (interleaved in pairs
    across the two PSUM scratch banks so LDWEIGHTS overlaps the previous
    matmul), freeing those banks before any attention matmul issues; the
    attention then runs 8 row-blocks wide with no tail sweep, overlapping
    the ACT weight pass which paces the second phase.
  * Row-sharding: each core owns N/8 output rows; each core recomputes
    X~ = nf @ [W | W@v0 | W@v1] for ALL rows from a replicated bf16 nf^T
    (collectives measured ~90us of barrier+skew on this fabric).
  * Per-core the j-axis is rotated host-side so the core's own rows come
    first: f0 (needed by every attention chunk) is computed from the
    first two streamed nf groups, with a K=1 matmul broadcasting the f0
    row to all 128 partitions. Softmax sums are order-invariant.
"""

import json
import os
import shutil
import tempfile

import numpy as np

# ----------------------------------------------------------------------------
# ACT table patch: refit exp's buckets to g(x)=exp(sigmoid(x)) for |x|<32,
# zero for x in (-128,-32], true exp kept for x in [32,128). Entry format
# (8 x fp32): [d0, d1, d2, d3, x0, 0, 0, 0], y = d0+d1 t+d2 t^2+d3 t^3,
# t = x - x0. Centers/ctrl/profile structure unchanged.
# ----------------------------------------------------------------------------

_ACT_SET = "exp_and_others"


def _g_target(x):
    return np.exp(1.0 / (1.0 + np.exp(-x)))


def _fit_cubic(f, lo, hi):
    x0 = 0.5 * (lo + hi)
    xs = x0 + 0.5 * (hi - lo) * np.cos(np.linspace(0, np.pi, 33))
    t = (xs - x0).astype(np.float64)
    A = np.stack([np.ones_like(t), t, t * t, t ** 3], axis=1)
    coef, *_ = np.linalg.lstsq(A, f(xs.astype(np.float64)), rcond=None)
    return coef, x0


def _patched_act_tables():
    """Write a patched copy of the pwp table dir; return act_info.json path."""
    from neuronxcc.driver.Job import Job

    src = os.path.join(Job.getPackageDir(), "pwp", "pwp_bin_trainium")
    dst = os.path.join(tempfile.gettempdir(), "gat_actpatch_v1")
    marker = os.path.join(dst, ".done")
    info = os.path.join(dst, "act_info.json")
    if os.path.exists(marker):
        return info
    shutil.rmtree(dst, ignore_errors=True)
    os.makedirs(dst)
    for f in os.listdir(src):
        shutil.copy(os.path.join(src, f), os.path.join(dst, f))
        os.chmod(os.path.join(dst, f), 0o644)

    bkt = np.fromfile(os.path.join(dst, f"{_ACT_SET}_bkt.bin"), dtype=np.float32)
    bkt = bkt.reshape(-1, 8).copy()
    prof = json.load(open(os.path.join(dst, f"{_ACT_SET}.json")))

    groups = {}
    for i in range(781):
        if i in (777, 778, 779, 780):
            continue
        x0 = float(bkt[i, 4])
        if x0 == 0.0:
            continue
        e = int(np.floor(np.log2(abs(x0))))
        groups.setdefault((np.sign(x0), e), []).append(i)
    for (sgn, e), idxs in groups.items():
        idxs.sort(key=lambda i: bkt[i, 4])
        centers = bkt[idxs, 4].astype(np.float64)
        w = float(np.min(np.diff(centers))) if len(idxs) > 1 else float(2.0 ** e)
        for i in idxs:
            x0 = float(bkt[i, 4])
            if x0 > 0 and e >= 5:
                continue          # keep true exp on [32, 128): elu epilogue
            if x0 < 0 and e >= 5:
                bkt[i, 0:4] = 0.0  # (-128, -32]: zero (unreachable margin)
                continue
            coef, _ = _fit_cubic(_g_target, x0 - w / 2, x0 + w / 2)
            bkt[i, 0:4] = coef.astype(np.float32)
    g0 = float(np.exp(0.5))
    for i in (777, 778):           # |x| < 2^-19 small-signal buckets
        bkt[i, 0:4] = [g0, 0.25 * g0, 0.5 * g0 * 0.25 ** 2, 0.0]
        bkt[i, 4] = 0.0
    for ent in prof["profile_meta_data"]:
        if ent["func_name"].startswith("exp_"):
            ent["fzero_result"] = int(np.float32(g0).view(np.uint32))

    bkt.astype(np.float32).tofile(os.path.join(dst, f"{_ACT_SET}_bkt.bin"))
    with open(os.path.join(dst, f"{_ACT_SET}.json"), "w") as fh:
        json.dump(prof, fh)
    open(marker, "w").close()
    return info


os.environ["BASS_ACT_ROOT_JSON_PATH"] = _patched_act_tables()

import concourse.bass as bass
import concourse.mybir as mybir
import concourse.tile as tile
from concourse.bass_utils import run_bass_kernel_spmd

# ----------------------------------------------------------------------------
# Workaround for "Too many sync wait commands": this walrus build accepts only
# ONE sync-wait per instruction. Post-pass: hoist surplus waits onto
# single-wait NOPs on the same engine, inserted immediately before the
# instruction (identical blocking semantics, per-engine order preserved).
# ----------------------------------------------------------------------------


def _split_multi_waits(nc):
    import bass_rust

    eng = {
        mybir.EngineType.PE: nc.tensor,
        mybir.EngineType.DVE: nc.vector,
        mybir.EngineType.Activation: nc.scalar,
        mybir.EngineType.Pool: nc.gpsimd,
        mybir.EngineType.SP: nc.sync,
    }
    for f in nc.m.functions:
        for blk in f.blocks:
            fixups = []  # (index, inst, waits)
            for idx, inst in enumerate(blk.instructions):
                si = inst.sync_info
                waits = list(si.on_wait) if si is not None and si.on_wait else []
                if len(waits) > 1 and inst.engine in eng:
                    fixups.append((idx, inst, waits))
            if not fixups:
                continue
            nops_by_idx = {}
            created = set()
            for idx, inst, waits in fixups:
                inst.sync_info.on_wait = [waits[-1]]
                nops = []
                for w in waits[:-1]:
                    nop = eng[inst.engine].nop(nofuse=True, hint="wait_split").ins
                    nop.sync_info = bass_rust.SyncInfo(on_wait=[w], on_update=[])
                    nops.append(nop)
                    created.add(id(nop))
                nops_by_idx[idx] = nops
            for b2 in f.blocks:
                b2.instructions[:] = [
                    i for i in b2.instructions if id(i) not in created
                ]
            new = []
            for idx, inst in enumerate(blk.instructions):
                new.extend(nops_by_idx.get(idx, ()))
                new.append(inst)
            blk.instructions[:] = new


# ----------------------------------------------------------------------------

F32 = mybir.dt.float32
BF16 = mybir.dt.bfloat16
AF = mybir.ActivationFunctionType
ALU = mybir.AluOpType

N, D_IN, D_OUT = 8192, 512, 256
M_CORES = 8
P = 128
EXP_SHIFT = 80.0  # elu exp computed as exp(u+80)*e^-80 (un-patched region)


def build_gat(n=N, d_in=D_IN, d_out=D_OUT, m_cores=M_CORES, grp=4, debug=False):
    """Per-core SPMD program. Inputs per core (j-axis rotated so own rows
    come first):
      nfT   [d_in, n]  bf16  -- node_features.T, columns rotated per core
      maskT [P, n_grp*grp*R] bf16 -- {1 unmasked, 0 masked}, tiled layout
      wext  [d_in, d_out+2] bf16 -- [W | W@v0 | W@v1]
    Output: out [R, d_out] f32 (this core's rows)."""
    R = n // m_cores
    NJ = n // P                  # 64 j-chunks
    IB = R // P                  # 8 output row-blocks
    DK = d_in // P               # 4 contraction chunks
    DEXT = d_out + 2
    DW = d_out + 1               # attention rhs: X | ones
    XW = d_out + 2               # xsb row: X | ones | f1
    n_grp = NJ // grp            # 16 groups
    JG = grp * P                 # 512 j per group

    nc = bass.Bass(num_devices=m_cores)
    # All inputs ship partition-major-tiled so each DMA has >=4KB contiguous
    # runs per partition (1KB-run APs measured ~75GB/s vs ~350 line rate):
    #   nfT[dp, g, kc, j]   = nf.T[kc*128+dp, g*512+j]
    #   maskT[dp, g, bb, i] = mask01.T[g*512+bb*128+dp, i]
    #   wext[dp, kc, c]     = [W|Wv0|Wv1][kc*128+dp, c]
    maskT = nc.declare_dram_parameter("maskT", [P, n_grp * grp * R], BF16, isOutput=False)
    nfT = nc.declare_dram_parameter("nfT", [P, n_grp * DK * JG], BF16, isOutput=False)
    wext = nc.declare_dram_parameter("wext", [P, DK * DEXT], BF16, isOutput=False)
    outp = nc.declare_dram_parameter("out", [R, d_out], F32, isOutput=True)

    with tile.TileContext(nc) as tc:
        with tc.tile_pool(name="persist", bufs=1) as persist, \
             tc.tile_pool(name="nfc", bufs=2) as nf_pool, \
             tc.tile_pool(name="mk", bufs=2) as mk_pool, \
             tc.tile_pool(name="pg", bufs=16) as p_pool, \
             tc.tile_pool(name="epi", bufs=2) as epi:

            wextb = persist.tile([P, DK, DEXT], BF16)
            nc.sync.dma_start(
                out=wextb,
                in_=bass.AP(wext, 0, [[DK * DEXT, P], [DEXT, DK], [1, DEXT]]),
            )
            # prewarm the (patched) exp table during the preamble
            warm = persist.tile([P, 1], F32)
            nc.gpsimd.memset(warm, 40.0)   # true-exp region, harmless
            nc.scalar.activation(out=warm, in_=warm, func=AF.Exp)

            # xsb row layout: [X (256) | ones (256) | f1 (257)]; the ones
            # column is re-memset per group after the 258-wide PSUM cast
            # (which drops f0-junk there); attention rhs reads cols 0:257.
            xsb = persist.tile([P, NJ, XW], BF16)
            f0rep = persist.tile([P, R], F32)
            f0flat = persist.tile([1, R], F32)
            ones1 = persist.tile([1, P], F32)
            nc.vector.memset(ones1, 1.0)
            b80 = persist.tile([P, 1], F32)
            nc.vector.memset(b80, EXP_SHIFT)

            pg_tiles = []
            mk_tiles = {}
            with tc.tile_pool(name="ps1", bufs=1, space="PSUM") as ps1:
                # 8 PSUM banks, fully subscribed:
                #   5 dedicated attention accumulators (row-blocks 0-4)
                #   1 shared bank, serially: f0 row psums -> f0 broadcast
                #     halves -> 6th accumulator (row-block 5)
                #   2 X~ tile slots; after the last X~ tile their slots
                #     become accumulators for row-blocks 6-7 (tail).
                # Row-blocks 0-5 accumulate DURING the X~/ACT streaming
                # phase (interleaved per group, 2 groups behind so PE never
                # waits on the mask+mult latency); 6-7 run as a short tail.
                acc = [
                    ps1.tile([P, DW], F32, tag=f"acc{ib}", name=f"acc{ib}")
                    for ib in range(5)
                ]
                NIB1 = 6

                def sh1(shape, name):
                    return ps1.tile(shape, F32, tag="sh1", name=name, bufs=1)

                def emit_attn(g, ibs):
                    for bb in range(grp):
                        chunk = g * grp + bb
                        for ib in ibs:
                            nc.tensor.matmul(
                                out=acc[ib],
                                lhsT=pg_tiles[g][:, bb, ib * P:(ib + 1) * P],
                                rhs=xsb[:, chunk, 0:DW],
                                start=(chunk == 0),
                                stop=(chunk == NJ - 1),
                            )

                def emit_weights(g):
                    """ACT pass + multiplicative {0,1} mask for group g. Must
                    only be emitted once f0rep's write has been emitted (reads
                    emitted before a tile's first write see garbage)."""
                    pg = pg_tiles[g]
                    # one ACT pass: p = g(f0_i + f1_j) = exp(sigmoid(z))
                    for bb in range(grp):
                        jc = g * grp + bb
                        nc.scalar.activation(
                            out=pg[:, bb, :],
                            in_=f0rep,
                            func=AF.Exp,
                            bias=xsb[:, jc, d_out + 1:d_out + 2],
                            scale=1.0,
                        )
                    # mask: one fused bf16 multiply (2x packed DVE mode);
                    # p >= 1 for unmasked entries so {0,1}-multiply is exact
                    mk = mk_tiles.pop(g)
                    nc.vector.tensor_tensor(
                        out=pg.rearrange("p g r -> p (g r)"),
                        in0=pg.rearrange("p g r -> p (g r)"),
                        in1=mk.rearrange("p g r -> p (g r)"),
                        op=ALU.mult,
                    )

                f0ps = [None, None]

                def emit_group_x(g):
                    """nf + mask DMAs, f0 rows (g<2), X~ matmuls + casts."""
                    nfc = nf_pool.tile([P, DK, JG], BF16, tag="nfc")
                    nc.sync.dma_start(
                        out=nfc,
                        in_=bass.AP(
                            nfT, g * DK * JG,
                            [[n_grp * DK * JG, P], [JG, DK], [1, JG]],
                        ),
                    )
                    # mask tile for this group: SWDGE on the idle GpSimd
                    # queue so its buffer-waits never stall the nf stream
                    mk = mk_pool.tile([P, grp, R], BF16, tag="mk")
                    mk_tiles[g] = mk
                    nc.gpsimd.dma_start(
                        out=mk,
                        in_=bass.AP(
                            maskT, g * grp * R,
                            [[n_grp * grp * R, P], [R, grp], [1, R]],
                        ),
                    )
                    # f0 row for own rows (groups 0-1 under rotation), via
                    # stationary wv0 column (M=1) -> row-form, no transpose
                    if g < 2:
                        f0ps[g] = sh1([1, JG], f"f0ps{g}")
                        for kc in range(DK):
                            nc.tensor.matmul(
                                out=f0ps[g],
                                lhsT=wextb[:, kc, d_out:d_out + 1],
                                rhs=nfc[:, kc, :],
                                start=(kc == 0),
                                stop=(kc == DK - 1),
                            )
                        # ScE copy: keeps the f0 chain off the busy DVE
                        nc.scalar.activation(
                            out=f0flat[:, g * JG:(g + 1) * JG], in_=f0ps[g],
                            func=AF.Copy,
                        )
                    pg = p_pool.tile([P, grp, R], BF16, tag="pg")
                    pg_tiles.append(pg)
                    # X~ for 4 chunks, as 2 interleaved pairs ping-ponging
                    # the two PSUM scratch banks (LDWEIGHTS of one matmul
                    # overlaps the other bank's matmul stream)
                    for bb in (0, 2):
                        xa = ps1.tile([P, DEXT], F32, tag="xps", bufs=2)
                        xb = ps1.tile([P, DEXT], F32, tag="xps", bufs=2)
                        for kc in range(DK):
                            nc.tensor.matmul(
                                out=xa,
                                lhsT=nfc[:, kc, bb * P:(bb + 1) * P],
                                rhs=wextb[:, kc, :],
                                start=(kc == 0),
                                stop=(kc == DK - 1),
                            )
                            nc.tensor.matmul(
                                out=xb,
                                lhsT=nfc[:, kc, (bb + 1) * P:(bb + 2) * P],
                                rhs=wextb[:, kc, :],
                                start=(kc == 0),
                                stop=(kc == DK - 1),
                            )
                        # 258-wide casts: X -> 0:256, f0-junk -> 256 (ones
                        # re-memset below), f1 -> 257 (ACT bias reads it)
                        jc = g * grp + bb
                        nc.vector.tensor_copy(
                            out=xsb[:, jc, 0:DEXT], in_=xa[:, 0:DEXT]
                        )
                        nc.vector.tensor_copy(
                            out=xsb[:, jc + 1, 0:DEXT], in_=xb[:, 0:DEXT]
                        )
                    # restore the ones column for this group's 4 chunks
                    nc.vector.memset(xsb[:, g * grp:(g + 1) * grp, d_out], 1.0)

                for g in range(n_grp):
                    emit_group_x(g)
                    if g == 1:
                        # broadcast f0 row to all partitions via K=1 matmul,
                        # two halves serially through the shared bank
                        for h in range(R // JG):
                            fb = sh1([P, JG], f"f0bc{h}")
                            nc.tensor.matmul(
                                out=fb,
                                lhsT=ones1,
                                rhs=f0flat[:, h * JG:(h + 1) * JG],
                                start=True, stop=True,
                            )
                            nc.scalar.activation(
                                out=f0rep[:, h * JG:(h + 1) * JG], in_=fb,
                                func=AF.Copy,
                            )
                        emit_weights(0)
                        emit_weights(1)
                    elif g >= 2:
                        emit_weights(g)
                    if g >= 2:
                        if g == 2:
                            acc.append(sh1([P, DW], "acc5"))
                        emit_attn(g - 2, range(NIB1))
                emit_attn(n_grp - 2, range(NIB1))
                emit_attn(n_grp - 1, range(NIB1))

                kexp = float(np.exp(-EXP_SHIFT))

                def emit_epilogue(ib):
                    # out_row = elu(num / rowsum); rowsum >= 1 always (every
                    # row has a neighbor and p >= 1), so no guard needed.
                    r = epi.tile([P, 1], F32, tag="r")
                    nc.vector.reciprocal(out=r, in_=acc[ib][:, d_out:DW])
                    u = epi.tile([P, d_out], F32, tag="u")
                    nc.vector.tensor_scalar(
                        out=u, in0=acc[ib][:, 0:d_out], scalar1=r, scalar2=None,
                        op0=ALU.mult,
                    )
                    rp = epi.tile([P, d_out], F32, tag="rp")
                    nc.vector.tensor_scalar(
                        out=rp, in0=u, scalar1=0.0, scalar2=-1.0,
                        op0=ALU.max, op1=ALU.add,
                    )
                    xm = epi.tile([P, d_out], F32, tag="xm")
                    nc.vector.tensor_scalar_min(xm, u, 0.0)
                    en = epi.tile([P, d_out], F32, tag="en")
                    nc.scalar.activation(out=en, in_=xm, func=AF.Exp, bias=b80)
                    res = epi.tile([P, d_out], F32, tag="res")
                    nc.vector.scalar_tensor_tensor(
                        out=res, in0=en, scalar=kexp, in1=rp,
                        op0=ALU.mult, op1=ALU.add,
                    )
                    nc.sync.dma_start(out=outp[ib * P:(ib + 1) * P, :], in_=res)

                # blocks 0-5 epilogue overlaps the 6-7 matmul tail
                for ib in range(NIB1):
                    emit_epilogue(ib)

                # tail: last two row-blocks through the freed X~ slots
                acc.append(ps1.tile([P, DEXT], F32, tag="xps", name="acc6",
                                    bufs=2)[:, 0:DW])
                acc.append(ps1.tile([P, DEXT], F32, tag="xps", name="acc7",
                                    bufs=2)[:, 0:DW])
                for chunk in range(NJ):
                    g, bb = divmod(chunk, grp)
                    for ib in (6, 7):
                        nc.tensor.matmul(
                            out=acc[ib],
                            lhsT=pg_tiles[g][:, bb, ib * P:(ib + 1) * P],
                            rhs=xsb[:, chunk, 0:DW],
                            start=(chunk == 0),
                            stop=(chunk == NJ - 1),
                        )
                emit_epilogue(6)
                emit_epilogue(7)

    _split_multi_waits(nc)
    return nc


_cached = {}

# Dev/test knobs (the grading harness just calls kernel(**inputs)):
_TRACE = False
_TMPDIR = None
_LAST_EXEC_NS = None
_LAST_RESULTS = None


def _get_program(n, d_in, d_out, m_cores):
    key = (n, d_in, d_out, m_cores)
    if key not in _cached:
        _cached[key] = build_gat(n, d_in, d_out, m_cores)
    return _cached[key]


def kernel(node_features, graph, W, v0, v1):
    import ml_dtypes

    node_features = np.asarray(node_features, dtype=np.float32)
    graph = np.asarray(graph)
    W = np.asarray(W, dtype=np.float32)
    v0 = np.asarray(v0, dtype=np.float32)
    v1 = np.asarray(v1, dtype=np.float32)

    n, d_in = node_features.shape
    d_out = W.shape[1]
    m = M_CORES
    R = n // m

    nc = _get_program(n, d_in, d_out, m)

    DK, n_grp, JG, grp = d_in // P, n // P // 4, 4 * P, 4
    wext = np.concatenate([W, W @ v0, W @ v1], axis=1).astype(ml_dtypes.bfloat16)
    wext_t = np.ascontiguousarray(
        wext.reshape(DK, P, d_out + 2).transpose(1, 0, 2).reshape(P, -1)
    )
    nfT = node_features.T.astype(ml_dtypes.bfloat16)
    mask01 = (graph != 0).astype(ml_dtypes.bfloat16)

    in_maps = []
    for c in range(m):
        rows = slice(c * R, (c + 1) * R)
        roll = -c * R
        nf_c = np.roll(nfT, roll, axis=1)          # [d_in, n], own cols first
        nf_t = np.ascontiguousarray(
            nf_c.reshape(DK, P, n_grp, JG).transpose(1, 2, 0, 3).reshape(P, -1)
        )
        m_c = np.roll(mask01[rows].T, roll, axis=0)    # [n, R]
        m_t = np.ascontiguousarray(
            m_c.reshape(n_grp, grp, P, R).transpose(2, 0, 1, 3).reshape(P, -1)
        )
        in_maps.append({"nfT": nf_t, "maskT": m_t, "wext": wext_t})
    global _LAST_EXEC_NS, _LAST_RESULTS
    res = run_bass_kernel_spmd(
        nc, in_maps, list(range(m)), trace=_TRACE, tmpdir=_TMPDIR
    )
    _LAST_EXEC_NS = res.exec_time_ns
    _LAST_RESULTS = res
    return np.concatenate([res.results[c]["out"] for c in range(m)], axis=0)


# revision 28
# speedup vs baseline: 1.1335x; 1.0554x over previous
"""GAT layer kernel for Trainium2, 8 NeuronCores.

Reference computation:
    X = node_features @ W            [N, DOUT]
    f0 = X @ v0 ; f1 = X @ v1       [N, 1]
    vals = sigmoid(f0 + f1.T) - 0.5
    alphas = softmax(where(graph != 0, vals, -inf), axis=1), masked to 0
    out = elu(alphas @ X)

Design notes:
  * softmax(sigmoid(z) - 0.5) == softmax(sigmoid(z)) (row-constant shift),
    so the softmax weights are w = exp(sigmoid(z)) up to a global scale.
  * w is evaluated in ONE activation pass through a patched ACT spline
    table: the `exp` buckets for |x| < 32 are refit to g(x)=exp(sigmoid(x))
    (bucket centers and ctrl tables unchanged, only cubic coefficients),
    while [32,128) keeps true exp so the elu epilogue can compute
    exp(u) = exp(u + 80) * e^-80 out of the un-patched region.
  * The adjacency mask ships as bf16 {0,1}, streamed by plain SWDGE DMA
    (tiled layout, 8KB contiguous runs per partition) on the otherwise-idle
    GpSimd queue, and applied multiplicatively with a single DVE
    tensor_tensor per group (bf16 2x packed mode). w >= 1 for unmasked
    entries so multiply-by-{0,1} is an exact mask.
  * softmax ratio: out_row = (sum_j p_ij X_j) / (sum_j p_ij); the row-sum
    comes free as a ones-column in the matmul rhs.
  * Two-phase schedule: ALL X~ chunks stream first (interleaved in pairs
    across the two PSUM scratch banks so LDWEIGHTS overlaps the previous
    matmul), freeing those banks before any attention matmul issues; the
    attention then runs 8 row-blocks wide with no tail sweep, overlapping
    the ACT weight pass which paces the second phase.
  * Row-sharding: each core owns N/8 output rows; each core recomputes
    X~ = nf @ [W | W@v0 | W@v1] for ALL rows from a replicated bf16 nf^T
    (collectives measured ~90us of barrier+skew on this fabric).
  * Per-core the j-axis is rotated host-side so the core's own rows come
    first: f0 (needed by every attention chunk) is computed from the
    first two streamed nf groups, with a K=1 matmul broadcasting the f0
    row to all 128 partitions. Softmax sums are order-invariant.
"""

import json
import os
import shutil
import tempfile

import numpy as np

# ----------------------------------------------------------------------------
# ACT table patch: refit exp's buckets to g(x)=exp(sigmoid(x)) for |x|<32,
# zero for x in (-128,-32], true exp kept for x in [32,128). Entry format
# (8 x fp32): [d0, d1, d2, d3, x0, 0, 0, 0], y = d0+d1 t+d2 t^2+d3 t^3,
# t = x - x0. Centers/ctrl/profile structure unchanged.
# ----------------------------------------------------------------------------

_ACT_SET = "exp_and_others"


def _g_target(x):
    return np.exp(1.0 / (1.0 + np.exp(-x)))


def _fit_cubic(f, lo, hi):
    x0 = 0.5 * (lo + hi)
    xs = x0 + 0.5 * (hi - lo) * np.cos(np.linspace(0, np.pi, 33))
    t = (xs - x0).astype(np.float64)
    A = np.stack([np.ones_like(t), t, t * t, t ** 3], axis=1)
    coef, *_ = np.linalg.lstsq(A, f(xs.astype(np.float64)), rcond=None)
    return coef, x0


def _patched_act_tables():
    """Write a patched copy of the pwp table dir; return act_info.json path."""
    from neuronxcc.driver.Job import Job

    src = os.path.join(Job.getPackageDir(), "pwp", "pwp_bin_trainium")
    dst = os.path.join(tempfile.gettempdir(), "gat_actpatch_v1")
    marker = os.path.join(dst, ".done")
    info = os.path.join(dst, "act_info.json")
    if os.path.exists(marker):
        return info
    shutil.rmtree(dst, ignore_errors=True)
    os.makedirs(dst)
    for f in os.listdir(src):
        shutil.copy(os.path.join(src, f), os.path.join(dst, f))
        os.chmod(os.path.join(dst, f), 0o644)

    bkt = np.fromfile(os.path.join(dst, f"{_ACT_SET}_bkt.bin"), dtype=np.float32)
    bkt = bkt.reshape(-1, 8).copy()
    prof = json.load(open(os.path.join(dst, f"{_ACT_SET}.json")))

    groups = {}
    for i in range(781):
        if i in (777, 778, 779, 780):
            continue
        x0 = float(bkt[i, 4])
        if x0 == 0.0:
            continue
        e = int(np.floor(np.log2(abs(x0))))
        groups.setdefault((np.sign(x0), e), []).append(i)
    for (sgn, e), idxs in groups.items():
        idxs.sort(key=lambda i: bkt[i, 4])
        centers = bkt[idxs, 4].astype(np.float64)
        w = float(np.min(np.diff(centers))) if len(idxs) > 1 else float(2.0 ** e)
        for i in idxs:
            x0 = float(bkt[i, 4])
            if x0 > 0 and e >= 5:
                continue          # keep true exp on [32, 128): elu epilogue
            if x0 < 0 and e >= 5:
                bkt[i, 0:4] = 0.0  # (-128, -32]: zero (unreachable margin)
                continue
            coef, _ = _fit_cubic(_g_target, x0 - w / 2, x0 + w / 2)
            bkt[i, 0:4] = coef.astype(np.float32)
    g0 = float(np.exp(0.5))
    for i in (777, 778):           # |x| < 2^-19 small-signal buckets
        bkt[i, 0:4] = [g0, 0.25 * g0, 0.5 * g0 * 0.25 ** 2, 0.0]
        bkt[i, 4] = 0.0
    for ent in prof["profile_meta_data"]:
        if ent["func_name"].startswith("exp_"):
            ent["fzero_result"] = int(np.float32(g0).view(np.uint32))

    bkt.astype(np.float32).tofile(os.path.join(dst, f"{_ACT_SET}_bkt.bin"))
    with open(os.path.join(dst, f"{_ACT_SET}.json"), "w") as fh:
        json.dump(prof, fh)
    open(marker, "w").close()
    return info


os.environ["BASS_ACT_ROOT_JSON_PATH"] = _patched_act_tables()

import concourse.bass as bass
import concourse.mybir as mybir
import concourse.tile as tile
from concourse.bass_utils import run_bass_kernel_spmd

# ----------------------------------------------------------------------------
# Workaround for "Too many sync wait commands": this walrus build accepts only
# ONE sync-wait per instruction. Post-pass: hoist surplus waits onto
# single-wait NOPs on the same engine, inserted immediately before the
# instruction (identical blocking semantics, per-engine order preserved).
# ----------------------------------------------------------------------------


def _split_multi_waits(nc):
    import bass_rust

    eng = {
        mybir.EngineType.PE: nc.tensor,
        mybir.EngineType.DVE: nc.vector,
        mybir.EngineType.Activation: nc.scalar,
        mybir.EngineType.Pool: nc.gpsimd,
        mybir.EngineType.SP: nc.sync,
    }
    for f in nc.m.functions:
        for blk in f.blocks:
            fixups = []  # (index, inst, waits)
            for idx, inst in enumerate(blk.instructions):
                si = inst.sync_info
                waits = list(si.on_wait) if si is not None and si.on_wait else []
                if len(waits) > 1 and inst.engine in eng:
                    fixups.append((idx, inst, waits))
            if not fixups:
                continue
            nops_by_idx = {}
            created = set()
            for idx, inst, waits in fixups:
                inst.sync_info.on_wait = [waits[-1]]
                nops = []
                for w in waits[:-1]:
                    nop = eng[inst.engine].nop(nofuse=True, hint="wait_split").ins
                    nop.sync_info = bass_rust.SyncInfo(on_wait=[w], on_update=[])
                    nops.append(nop)
                    created.add(id(nop))
                nops_by_idx[idx] = nops
            for b2 in f.blocks:
                b2.instructions[:] = [
                    i for i in b2.instructions if id(i) not in created
                ]
            new = []
            for idx, inst in enumerate(blk.instructions):
                new.extend(nops_by_idx.get(idx, ()))
                new.append(inst)
            blk.instructions[:] = new


# ----------------------------------------------------------------------------

F32 = mybir.dt.float32
BF16 = mybir.dt.bfloat16
AF = mybir.ActivationFunctionType
ALU = mybir.AluOpType

N, D_IN, D_OUT = 8192, 512, 256
M_CORES = 8
P = 128
EXP_SHIFT = 80.0  # elu exp computed as exp(u+80)*e^-80 (un-patched region)


def build_gat(n=N, d_in=D_IN, d_out=D_OUT, m_cores=M_CORES, grp=4, debug=False):
    """Per-core SPMD program. Inputs per core (j-axis rotated so own rows
    come first):
      nfT   [d_in, n]  bf16  -- node_features.T, columns rotated per core
      maskT [P, n_grp*grp*R] bf16 -- {1 unmasked, 0 masked}, tiled layout
      wext  [d_in, d_out+2] bf16 -- [W | W@v0 | W@v1]
    Output: out [R, d_out] f32 (this core's rows)."""
    R = n // m_cores
    NJ = n // P                  # 64 j-chunks
    IB = R // P                  # 8 output row-blocks
    DK = d_in // P               # 4 contraction chunks
    DEXT = d_out + 2
    DW = d_out + 1               # attention rhs: X | ones
    XW = d_out + 2               # xsb row: X | ones | f1
    n_grp = NJ // grp            # 16 groups
    JG = grp * P                 # 512 j per group

    nc = bass.Bass(num_devices=m_cores)
    # All inputs ship partition-major-tiled so each DMA has >=4KB contiguous
    # runs per partition (1KB-run APs measured ~75GB/s vs ~350 line rate):
    #   nfT[dp, g, kc, j]   = nf.T[kc*128+dp, g*512+j]
    #   maskT[dp, g, bb, i] = mask01.T[g*512+bb*128+dp, i]
    #   wext[dp, kc, c]     = [W|Wv0|Wv1][kc*128+dp, c]
    maskT = nc.declare_dram_parameter("maskT", [P, n_grp * grp * R], BF16, isOutput=False)
    nfT = nc.declare_dram_parameter("nfT", [P, n_grp * DK * JG], BF16, isOutput=False)
    wext = nc.declare_dram_parameter("wext", [P, DK * DEXT], BF16, isOutput=False)
    outp = nc.declare_dram_parameter("out", [R, d_out], F32, isOutput=True)

    with tile.TileContext(nc) as tc:
        with tc.tile_pool(name="persist", bufs=1) as persist, \
             tc.tile_pool(name="nfc", bufs=2) as nf_pool, \
             tc.tile_pool(name="mk", bufs=2) as mk_pool, \
             tc.tile_pool(name="pg", bufs=16) as p_pool, \
             tc.tile_pool(name="epi", bufs=2) as epi:

            wextb = persist.tile([P, DK, DEXT], BF16)
            nc.sync.dma_start(
                out=wextb,
                in_=bass.AP(wext, 0, [[DK * DEXT, P], [DEXT, DK], [1, DEXT]]),
            )
            # prewarm the (patched) exp table during the preamble
            warm = persist.tile([P, 1], F32)
            nc.gpsimd.memset(warm, 40.0)   # true-exp region, harmless
            nc.scalar.activation(out=warm, in_=warm, func=AF.Exp)

            # xsb row layout: [X (256) | ones (256) | f1 (257)]; the ones
            # column is re-memset per group after the 258-wide PSUM cast
            # (which drops f0-junk there); attention rhs reads cols 0:257.
            xsb = persist.tile([P, NJ, XW], BF16)
            f0rep = persist.tile([P, R], F32)
            f0flat = persist.tile([1, R], F32)
            ones1 = persist.tile([1, P], F32)
            nc.vector.memset(ones1, 1.0)
            b80 = persist.tile([P, 1], F32)
            nc.vector.memset(b80, EXP_SHIFT)

            pg_tiles = []
            mk_tiles = {}
            with tc.tile_pool(name="ps1", bufs=1, space="PSUM") as ps1:
                # 8 PSUM banks, fully subscribed:
                #   5 dedicated attention accumulators (row-blocks 0-4)
                #   1 shared bank, serially: f0 row psums -> f0 broadcast
                #     halves -> 6th accumulator (row-block 5)
                #   2 X~ tile slots; after the last X~ tile their slots
                #     become accumulators for row-blocks 6-7 (tail).
                # Row-blocks 0-5 accumulate DURING the X~/ACT streaming
                # phase (interleaved per group, 2 groups behind so PE never
                # waits on the mask+mult latency); 6-7 run as a short tail.
                acc = [
                    ps1.tile([P, DW], F32, tag=f"acc{ib}", name=f"acc{ib}")
                    for ib in range(5)
                ]
                NIB1 = 6

                def sh1(shape, name):
                    return ps1.tile(shape, F32, tag="sh1", name=name, bufs=1)

                def emit_attn(g, ibs):
                    for bb in range(grp):
                        chunk = g * grp + bb
                        for ib in ibs:
                            nc.tensor.matmul(
                                out=acc[ib],
                                lhsT=pg_tiles[g][:, bb, ib * P:(ib + 1) * P],
                                rhs=xsb[:, chunk, 0:DW],
                                start=(chunk == 0),
                                stop=(chunk == NJ - 1),
                            )

                def emit_weights(g):
                    """ACT pass + multiplicative {0,1} mask for group g. Must
                    only be emitted once f0rep's write has been emitted (reads
                    emitted before a tile's first write see garbage)."""
                    pg = pg_tiles[g]
                    # one ACT pass: p = g(f0_i + f1_j) = exp(sigmoid(z))
                    for bb in range(grp):
                        jc = g * grp + bb
                        nc.scalar.activation(
                            out=pg[:, bb, :],
                            in_=f0rep,
                            func=AF.Exp,
                            bias=xsb[:, jc, d_out + 1:d_out + 2],
                            scale=1.0,
                        )
                    # mask: one fused bf16 multiply (2x packed DVE mode);
                    # p >= 1 for unmasked entries so {0,1}-multiply is exact
                    mk = mk_tiles.pop(g)
                    nc.vector.tensor_tensor(
                        out=pg.rearrange("p g r -> p (g r)"),
                        in0=pg.rearrange("p g r -> p (g r)"),
                        in1=mk.rearrange("p g r -> p (g r)"),
                        op=ALU.mult,
                    )

                f0ps = [None, None]

                def emit_group_x(g):
                    """nf + mask DMAs, f0 rows (g<2), X~ matmuls + casts."""
                    nfc = nf_pool.tile([P, DK, JG], BF16, tag="nfc")
                    nc.sync.dma_start(
                        out=nfc,
                        in_=bass.AP(
                            nfT, g * DK * JG,
                            [[n_grp * DK * JG, P], [JG, DK], [1, JG]],
                        ),
                    )
                    # mask tile for this group: SWDGE on the idle GpSimd
                    # queue so its buffer-waits never stall the nf stream
                    mk = mk_pool.tile([P, grp, R], BF16, tag="mk")
                    mk_tiles[g] = mk
                    nc.gpsimd.dma_start(
                        out=mk,
                        in_=bass.AP(
                            maskT, g * grp * R,
                            [[n_grp * grp * R, P], [R, grp], [1, R]],
                        ),
                    )
                    # f0 row for own rows (groups 0-1 under rotation), via
                    # stationary wv0 column (M=1) -> row-form, no transpose
                    if g < 2:
                        f0ps[g] = sh1([1, JG], f"f0ps{g}")
                        for kc in range(DK):
                            nc.tensor.matmul(
                                out=f0ps[g],
                                lhsT=wextb[:, kc, d_out:d_out + 1],
                                rhs=nfc[:, kc, :],
                                start=(kc == 0),
                                stop=(kc == DK - 1),
                            )
                        # ScE copy: keeps the f0 chain off the busy DVE
                        nc.scalar.activation(
                            out=f0flat[:, g * JG:(g + 1) * JG], in_=f0ps[g],
                            func=AF.Copy,
                        )
                    pg = p_pool.tile([P, grp, R], BF16, tag="pg")
                    pg_tiles.append(pg)
                    # X~ for 4 chunks, as 2 interleaved pairs ping-ponging
                    # the two PSUM scratch banks (LDWEIGHTS of one matmul
                    # overlaps the other bank's matmul stream)
                    for bb in (0, 2):
                        xa = ps1.tile([P, DEXT], F32, tag="xps", bufs=2)
                        xb = ps1.tile([P, DEXT], F32, tag="xps", bufs=2)
                        for kc in range(DK):
                            nc.tensor.matmul(
                                out=xa,
                                lhsT=nfc[:, kc, bb * P:(bb + 1) * P],
                                rhs=wextb[:, kc, :],
                                start=(kc == 0),
                                stop=(kc == DK - 1),
                            )
                            nc.tensor.matmul(
                                out=xb,
                                lhsT=nfc[:, kc, (bb + 1) * P:(bb + 2) * P],
                                rhs=wextb[:, kc, :],
                                start=(kc == 0),
                                stop=(kc == DK - 1),
                            )
                        # 258-wide casts: X -> 0:256, f0-junk -> 256 (ones
                        # re-memset below), f1 -> 257 (ACT bias reads it)
                        jc = g * grp + bb
                        nc.vector.tensor_copy(
                            out=xsb[:, jc, 0:DEXT], in_=xa[:, 0:DEXT]
                        )
                        nc.vector.tensor_copy(
                            out=xsb[:, jc + 1, 0:DEXT], in_=xb[:, 0:DEXT]
                        )
                    # restore the ones column for this group's 4 chunks
                    nc.vector.memset(xsb[:, g * grp:(g + 1) * grp, d_out], 1.0)

                for g in range(n_grp):
                    emit_group_x(g)
                    if g == 1:
                        # broadcast f0 row to all partitions via K=1 matmul,
                        # two halves serially through the shared bank
                        for h in range(R // JG):
                            fb = sh1([P, JG], f"f0bc{h}")
                            nc.tensor.matmul(
                                out=fb,
                                lhsT=ones1,
                                rhs=f0flat[:, h * JG:(h + 1) * JG],
                                start=True, stop=True,
                            )
                            nc.scalar.activation(
                                out=f0rep[:, h * JG:(h + 1) * JG], in_=fb,
                                func=AF.Copy,
                            )
                        emit_weights(0)
                        emit_weights(1)
                    elif g >= 2:
                        emit_weights(g)
                    if g >= 3:
                        if g == 3:
                            acc.append(sh1([P, DW], "acc5"))
                        emit_attn(g - 3, range(NIB1))
                emit_attn(n_grp - 3, range(NIB1))
                emit_attn(n_grp - 2, range(NIB1))
                emit_attn(n_grp - 1, range(NIB1))

                kexp = float(np.exp(-EXP_SHIFT))

                def emit_epilogue(ib):
                    # out_row = elu(num / rowsum); rowsum >= 1 always (every
                    # row has a neighbor and p >= 1), so no guard needed.
                    r = epi.tile([P, 1], F32, tag="r")
                    nc.vector.reciprocal(out=r, in_=acc[ib][:, d_out:DW])
                    u = epi.tile([P, d_out], F32, tag="u")
                    nc.vector.tensor_scalar(
                        out=u, in0=acc[ib][:, 0:d_out], scalar1=r, scalar2=None,
                        op0=ALU.mult,
                    )
                    rp = epi.tile([P, d_out], F32, tag="rp")
                    nc.vector.tensor_scalar(
                        out=rp, in0=u, scalar1=0.0, scalar2=-1.0,
                        op0=ALU.max, op1=ALU.add,
                    )
                    xm = epi.tile([P, d_out], F32, tag="xm")
                    nc.vector.tensor_scalar_min(xm, u, 0.0)
                    en = epi.tile([P, d_out], F32, tag="en")
                    nc.scalar.activation(out=en, in_=xm, func=AF.Exp, bias=b80)
                    res = epi.tile([P, d_out], F32, tag="res")
                    nc.vector.scalar_tensor_tensor(
                        out=res, in0=en, scalar=kexp, in1=rp,
                        op0=ALU.mult, op1=ALU.add,
                    )
                    nc.sync.dma_start(out=outp[ib * P:(ib + 1) * P, :], in_=res)

                # blocks 0-5 epilogue overlaps the 6-7 matmul tail
                for ib in range(NIB1):
                    emit_epilogue(ib)

                # tail: last two row-blocks through the freed X~ slots
                acc.append(ps1.tile([P, DEXT], F32, tag="xps", name="acc6",
                                    bufs=2)[:, 0:DW])
                acc.append(ps1.tile([P, DEXT], F32, tag="xps", name="acc7",
                                    bufs=2)[:, 0:DW])
                for chunk in range(NJ):
                    g, bb = divmod(chunk, grp)
                    for ib in (6, 7):
                        nc.tensor.matmul(
                            out=acc[ib],
                            lhsT=pg_tiles[g][:, bb, ib * P:(ib + 1) * P],
                            rhs=xsb[:, chunk, 0:DW],
                            start=(chunk == 0),
                            stop=(chunk == NJ - 1),
                        )
                emit_epilogue(6)
                emit_epilogue(7)

    _split_multi_waits(nc)
    return nc


_cached = {}

# Dev/test knobs (the grading harness just calls kernel(**inputs)):
_TRACE = False
_TMPDIR = None
_LAST_EXEC_NS = None
_LAST_RESULTS = None


def _get_program(n, d_in, d_out, m_cores):
    key = (n, d_in, d_out, m_cores)
    if key not in _cached:
        _cached[key] = build_gat(n, d_in, d_out, m_cores)
    return _cached[key]


def kernel(node_features, graph, W, v0, v1):
    import ml_dtypes

    node_features = np.asarray(node_features, dtype=np.float32)
    graph = np.asarray(graph)
    W = np.asarray(W, dtype=np.float32)
    v0 = np.asarray(v0, dtype=np.float32)
    v1 = np.asarray(v1, dtype=np.float32)

    n, d_in = node_features.shape
    d_out = W.shape[1]
    m = M_CORES
    R = n // m

    nc = _get_program(n, d_in, d_out, m)

    DK, n_grp, JG, grp = d_in // P, n // P // 4, 4 * P, 4
    wext = np.concatenate([W, W @ v0, W @ v1], axis=1).astype(ml_dtypes.bfloat16)
    wext_t = np.ascontiguousarray(
        wext.reshape(DK, P, d_out + 2).transpose(1, 0, 2).reshape(P, -1)
    )
    nfT = node_features.T.astype(ml_dtypes.bfloat16)
    mask01 = (graph != 0).astype(ml_dtypes.bfloat16)

    in_maps = []
    for c in range(m):
        rows = slice(c * R, (c + 1) * R)
        roll = -c * R
        nf_c = np.roll(nfT, roll, axis=1)          # [d_in, n], own cols first
        nf_t = np.ascontiguousarray(
            nf_c.reshape(DK, P, n_grp, JG).transpose(1, 2, 0, 3).reshape(P, -1)
        )
        m_c = np.roll(mask01[rows].T, roll, axis=0)    # [n, R]
        m_t = np.ascontiguousarray(
            m_c.reshape(n_grp, grp, P, R).transpose(2, 0, 1, 3).reshape(P, -1)
        )
        in_maps.append({"nfT": nf_t, "maskT": m_t, "wext": wext_t})
    global _LAST_EXEC_NS, _LAST_RESULTS
    res = run_bass_kernel_spmd(
        nc, in_maps, list(range(m)), trace=_TRACE, tmpdir=_TMPDIR
    )
    _LAST_EXEC_NS = res.exec_time_ns
    _LAST_RESULTS = res
    return np.concatenate([res.results[c]["out"] for c in range(m)], axis=0)


# revision 30
# speedup vs baseline: 1.1340x; 1.0004x over previous
"""GAT layer kernel for Trainium2, 8 NeuronCores.

Reference computation:
    X = node_features @ W            [N, DOUT]
    f0 = X @ v0 ; f1 = X @ v1       [N, 1]
    vals = sigmoid(f0 + f1.T) - 0.5
    alphas = softmax(where(graph != 0, vals, -inf), axis=1), masked to 0
    out = elu(alphas @ X)

Design notes:
  * softmax(sigmoid(z) - 0.5) == softmax(sigmoid(z)) (row-constant shift),
    so the softmax weights are w = exp(sigmoid(z)) up to a global scale.
  * w is evaluated in ONE activation pass through a patched ACT spline
    table: the `exp` buckets for |x| < 32 are refit to g(x)=exp(sigmoid(x))
    (bucket centers and ctrl tables unchanged, only cubic coefficients),
    while [32,128) keeps true exp so the elu epilogue can compute
    exp(u) = exp(u + 80) * e^-80 out of the un-patched region.
  * The adjacency mask ships as bf16 {0,1}, streamed by plain SWDGE DMA
    (tiled layout, 8KB contiguous runs per partition) on the otherwise-idle
    GpSimd queue, and applied multiplicatively with a single DVE
    tensor_tensor per group (bf16 2x packed mode). w >= 1 for unmasked
    entries so multiply-by-{0,1} is an exact mask.
  * softmax ratio: out_row = (sum_j p_ij X_j) / (sum_j p_ij); the row-sum
    comes free as a ones-column in the matmul rhs.
  * Two-phase schedule: ALL X~ chunks stream first (interleaved in pairs
    across the two PSUM scratch banks so LDWEIGHTS overlaps the previous
    matmul), freeing those banks before any attention matmul issues; the
    attention then runs 8 row-blocks wide with no tail sweep, overlapping
    the ACT weight pass which paces the second phase.
  * Row-sharding: each core owns N/8 output rows; each core recomputes
    X~ = nf @ [W | W@v0 | W@v1] for ALL rows from a replicated bf16 nf^T
    (collectives measured ~90us of barrier+skew on this fabric).
  * Per-core the j-axis is rotated host-side so the core's own rows come
    first: f0 (needed by every attention chunk) is computed from the
    first two streamed nf groups, with a K=1 matmul broadcasting the f0
    row to all 128 partitions. Softmax sums are order-invariant.
"""

import json
import os
import shutil
import tempfile

import numpy as np

# ----------------------------------------------------------------------------
# ACT table patch: refit exp's buckets to g(x)=exp(sigmoid(x)) for |x|<32,
# zero for x in (-128,-32], true exp kept for x in [32,128). Entry format
# (8 x fp32): [d0, d1, d2, d3, x0, 0, 0, 0], y = d0+d1 t+d2 t^2+d3 t^3,
# t = x - x0. Centers/ctrl/profile structure unchanged.
# ----------------------------------------------------------------------------

_ACT_SET = "exp_and_others"


def _g_target(x):
    return np.exp(1.0 / (1.0 + np.exp(-x)))


def _fit_cubic(f, lo, hi):
    x0 = 0.5 * (lo + hi)
    xs = x0 + 0.5 * (hi - lo) * np.cos(np.linspace(0, np.pi, 33))
    t = (xs - x0).astype(np.float64)
    A = np.stack([np.ones_like(t), t, t * t, t ** 3], axis=1)
    coef, *_ = np.linalg.lstsq(A, f(xs.astype(np.float64)), rcond=None)
    return coef, x0


def _patched_act_tables():
    """Write a patched copy of the pwp table dir; return act_info.json path."""
    from neuronxcc.driver.Job import Job

    src = os.path.join(Job.getPackageDir(), "pwp", "pwp_bin_trainium")
    dst = os.path.join(tempfile.gettempdir(), "gat_actpatch_v1")
    marker = os.path.join(dst, ".done")
    info = os.path.join(dst, "act_info.json")
    if os.path.exists(marker):
        return info
    shutil.rmtree(dst, ignore_errors=True)
    os.makedirs(dst)
    for f in os.listdir(src):
        shutil.copy(os.path.join(src, f), os.path.join(dst, f))
        os.chmod(os.path.join(dst, f), 0o644)

    bkt = np.fromfile(os.path.join(dst, f"{_ACT_SET}_bkt.bin"), dtype=np.float32)
    bkt = bkt.reshape(-1, 8).copy()
    prof = json.load(open(os.path.join(dst, f"{_ACT_SET}.json")))

    groups = {}
    for i in range(781):
        if i in (777, 778, 779, 780):
            continue
        x0 = float(bkt[i, 4])
        if x0 == 0.0:
            continue
        e = int(np.floor(np.log2(abs(x0))))
        groups.setdefault((np.sign(x0), e), []).append(i)
    for (sgn, e), idxs in groups.items():
        idxs.sort(key=lambda i: bkt[i, 4])
        centers = bkt[idxs, 4].astype(np.float64)
        w = float(np.min(np.diff(centers))) if len(idxs) > 1 else float(2.0 ** e)
        for i in idxs:
            x0 = float(bkt[i, 4])
            if x0 > 0 and e >= 5:
                continue          # keep true exp on [32, 128): elu epilogue
            if x0 < 0 and e >= 5:
                bkt[i, 0:4] = 0.0  # (-128, -32]: zero (unreachable margin)
                continue
            coef, _ = _fit_cubic(_g_target, x0 - w / 2, x0 + w / 2)
            bkt[i, 0:4] = coef.astype(np.float32)
    g0 = float(np.exp(0.5))
    for i in (777, 778):           # |x| < 2^-19 small-signal buckets
        bkt[i, 0:4] = [g0, 0.25 * g0, 0.5 * g0 * 0.25 ** 2, 0.0]
        bkt[i, 4] = 0.0
    for ent in prof["profile_meta_data"]:
        if ent["func_name"].startswith("exp_"):
            ent["fzero_result"] = int(np.float32(g0).view(np.uint32))

    bkt.astype(np.float32).tofile(os.path.join(dst, f"{_ACT_SET}_bkt.bin"))
    with open(os.path.join(dst, f"{_ACT_SET}.json"), "w") as fh:
        json.dump(prof, fh)
    open(marker, "w").close()
    return info


os.environ["BASS_ACT_ROOT_JSON_PATH"] = _patched_act_tables()

import concourse.bass as bass
import concourse.mybir as mybir
import concourse.tile as tile
from concourse.bass_utils import run_bass_kernel_spmd

# ----------------------------------------------------------------------------
# Workaround for "Too many sync wait commands": this walrus build accepts only
# ONE sync-wait per instruction. Post-pass: hoist surplus waits onto
# single-wait NOPs on the same engine, inserted immediately before the
# instruction (identical blocking semantics, per-engine order preserved).
# ----------------------------------------------------------------------------


def _split_multi_waits(nc):
    import bass_rust

    eng = {
        mybir.EngineType.PE: nc.tensor,
        mybir.EngineType.DVE: nc.vector,
        mybir.EngineType.Activation: nc.scalar,
        mybir.EngineType.Pool: nc.gpsimd,
        mybir.EngineType.SP: nc.sync,
    }
    for f in nc.m.functions:
        for blk in f.blocks:
            fixups = []  # (index, inst, waits)
            for idx, inst in enumerate(blk.instructions):
                si = inst.sync_info
                waits = list(si.on_wait) if si is not None and si.on_wait else []
                if len(waits) > 1 and inst.engine in eng:
                    fixups.append((idx, inst, waits))
            if not fixups:
                continue
            nops_by_idx = {}
            created = set()
            for idx, inst, waits in fixups:
                inst.sync_info.on_wait = [waits[-1]]
                nops = []
                for w in waits[:-1]:
                    nop = eng[inst.engine].nop(nofuse=True, hint="wait_split").ins
                    nop.sync_info = bass_rust.SyncInfo(on_wait=[w], on_update=[])
                    nops.append(nop)
                    created.add(id(nop))
                nops_by_idx[idx] = nops
            for b2 in f.blocks:
                b2.instructions[:] = [
                    i for i in b2.instructions if id(i) not in created
                ]
            new = []
            for idx, inst in enumerate(blk.instructions):
                new.extend(nops_by_idx.get(idx, ()))
                new.append(inst)
            blk.instructions[:] = new


# ----------------------------------------------------------------------------

F32 = mybir.dt.float32
BF16 = mybir.dt.bfloat16
AF = mybir.ActivationFunctionType
ALU = mybir.AluOpType

N, D_IN, D_OUT = 8192, 512, 256
M_CORES = 8
P = 128
EXP_SHIFT = 80.0  # elu exp computed as exp(u+80)*e^-80 (un-patched region)


def build_gat(n=N, d_in=D_IN, d_out=D_OUT, m_cores=M_CORES, grp=4, debug=False):
    """Per-core SPMD program. Inputs per core (j-axis rotated so own rows
    come first):
      nfT   [d_in, n]  bf16  -- node_features.T, columns rotated per core
      maskT [P, n_grp*grp*R] bf16 -- {1 unmasked, 0 masked}, tiled layout
      wext  [d_in, d_out+2] bf16 -- [W | W@v0 | W@v1]
    Output: out [R, d_out] f32 (this core's rows)."""
    R = n // m_cores
    NJ = n // P                  # 64 j-chunks
    IB = R // P                  # 8 output row-blocks
    DK = d_in // P               # 4 contraction chunks
    DEXT = d_out + 2
    DW = d_out + 1               # attention rhs: X | ones
    XW = d_out + 2               # xsb row: X | ones | f1
    n_grp = NJ // grp            # 16 groups
    JG = grp * P                 # 512 j per group

    nc = bass.Bass(num_devices=m_cores)
    # All inputs ship partition-major-tiled so each DMA has >=4KB contiguous
    # runs per partition (1KB-run APs measured ~75GB/s vs ~350 line rate):
    #   nfT[dp, g, kc, j]   = nf.T[kc*128+dp, g*512+j]
    #   maskT[dp, g, bb, i] = mask01.T[g*512+bb*128+dp, i]
    #   wext[dp, kc, c]     = [W|Wv0|Wv1][kc*128+dp, c]
    maskT = nc.declare_dram_parameter("maskT", [P, n_grp * grp * R], BF16, isOutput=False)
    nfT = nc.declare_dram_parameter("nfT", [P, n_grp * DK * JG], BF16, isOutput=False)
    wext = nc.declare_dram_parameter("wext", [P, DK * DEXT], BF16, isOutput=False)
    outp = nc.declare_dram_parameter("out", [R, d_out], F32, isOutput=True)

    with tile.TileContext(nc) as tc:
        with tc.tile_pool(name="persist", bufs=1) as persist, \
             tc.tile_pool(name="nfc", bufs=2) as nf_pool, \
             tc.tile_pool(name="mk", bufs=2) as mk_pool, \
             tc.tile_pool(name="pg", bufs=16) as p_pool, \
             tc.tile_pool(name="epi", bufs=2) as epi:

            wextb = persist.tile([P, DK, DEXT], BF16)
            nc.sync.dma_start(
                out=wextb,
                in_=bass.AP(wext, 0, [[DK * DEXT, P], [DEXT, DK], [1, DEXT]]),
            )
            # prewarm the (patched) exp table during the preamble
            warm = persist.tile([P, 1], F32)
            nc.gpsimd.memset(warm, 40.0)   # true-exp region, harmless
            nc.scalar.activation(out=warm, in_=warm, func=AF.Exp)

            # xsb row layout: [X (256) | ones (256) | f1 (257)]; the ones
            # column is re-memset per group after the 258-wide PSUM cast
            # (which drops f0-junk there); attention rhs reads cols 0:257.
            xsb = persist.tile([P, NJ, XW], BF16)
            f0rep = persist.tile([P, R], F32)
            f0flat = persist.tile([1, R], F32)
            ones1 = persist.tile([1, P], F32)
            nc.vector.memset(ones1, 1.0)
            b80 = persist.tile([P, 1], F32)
            nc.vector.memset(b80, EXP_SHIFT)

            pg_tiles = []
            mk_tiles = {}
            with tc.tile_pool(name="ps1", bufs=1, space="PSUM") as ps1:
                # 8 PSUM banks, fully subscribed:
                #   5 dedicated attention accumulators (row-blocks 0-4)
                #   1 shared bank, serially: f0 row psums -> f0 broadcast
                #     halves -> 6th accumulator (row-block 5)
                #   2 X~ tile slots; after the last X~ tile their slots
                #     become accumulators for row-blocks 6-7 (tail).
                # Row-blocks 0-5 accumulate DURING the X~/ACT streaming
                # phase (interleaved per group, 2 groups behind so PE never
                # waits on the mask+mult latency); 6-7 run as a short tail.
                acc = [
                    ps1.tile([P, DW], F32, tag=f"acc{ib}", name=f"acc{ib}")
                    for ib in range(5)
                ]
                NIB1 = 6

                def sh1(shape, name):
                    return ps1.tile(shape, F32, tag="sh1", name=name, bufs=1)

                def emit_attn(g, ibs):
                    for bb in range(grp):
                        chunk = g * grp + bb
                        for ib in ibs:
                            nc.tensor.matmul(
                                out=acc[ib],
                                lhsT=pg_tiles[g][:, bb, ib * P:(ib + 1) * P],
                                rhs=xsb[:, chunk, 0:DW],
                                start=(chunk == 0),
                                stop=(chunk == NJ - 1),
                            )

                def emit_weights(g):
                    """ACT pass + multiplicative {0,1} mask for group g. Must
                    only be emitted once f0rep's write has been emitted (reads
                    emitted before a tile's first write see garbage)."""
                    pg = pg_tiles[g]
                    # one ACT pass: p = g(f0_i + f1_j) = exp(sigmoid(z))
                    for bb in range(grp):
                        jc = g * grp + bb
                        nc.scalar.activation(
                            out=pg[:, bb, :],
                            in_=f0rep,
                            func=AF.Exp,
                            bias=xsb[:, jc, d_out + 1:d_out + 2],
                            scale=1.0,
                        )
                    # mask: one fused bf16 multiply (2x packed DVE mode);
                    # p >= 1 for unmasked entries so {0,1}-multiply is exact
                    mk = mk_tiles.pop(g)
                    nc.vector.tensor_tensor(
                        out=pg.rearrange("p g r -> p (g r)"),
                        in0=pg.rearrange("p g r -> p (g r)"),
                        in1=mk.rearrange("p g r -> p (g r)"),
                        op=ALU.mult,
                    )

                f0ps = [None, None]

                def emit_group_x(g):
                    """nf + mask DMAs, f0 rows (g<2), X~ matmuls + casts."""
                    nfc = nf_pool.tile([P, DK, JG], BF16, tag="nfc")
                    nc.sync.dma_start(
                        out=nfc,
                        in_=bass.AP(
                            nfT, g * DK * JG,
                            [[n_grp * DK * JG, P], [JG, DK], [1, JG]],
                        ),
                    )
                    # mask tile for this group: SWDGE on the idle GpSimd
                    # queue so its buffer-waits never stall the nf stream
                    mk = mk_pool.tile([P, grp, R], BF16, tag="mk")
                    mk_tiles[g] = mk
                    nc.gpsimd.dma_start(
                        out=mk,
                        in_=bass.AP(
                            maskT, g * grp * R,
                            [[n_grp * grp * R, P], [R, grp], [1, R]],
                        ),
                    )
                    # f0 row for own rows (groups 0-1 under rotation), via
                    # stationary wv0 column (M=1) -> row-form, no transpose
                    if g < 2:
                        f0ps[g] = sh1([1, JG], f"f0ps{g}")
                        for kc in range(DK):
                            nc.tensor.matmul(
                                out=f0ps[g],
                                lhsT=wextb[:, kc, d_out:d_out + 1],
                                rhs=nfc[:, kc, :],
                                start=(kc == 0),
                                stop=(kc == DK - 1),
                            )
                        # ScE copy: keeps the f0 chain off the busy DVE
                        nc.scalar.activation(
                            out=f0flat[:, g * JG:(g + 1) * JG], in_=f0ps[g],
                            func=AF.Copy,
                        )
                    if g == 1:
                        # broadcast f0 BEFORE X~(1) in the PE stream: the
                        # first ACT then waits only on group 0's casts, not
                        # on 32 cold X~ matmuls queued ahead of fb
                        for h in range(R // JG):
                            fb = sh1([P, JG], f"f0bc{h}")
                            nc.tensor.matmul(
                                out=fb,
                                lhsT=ones1,
                                rhs=f0flat[:, h * JG:(h + 1) * JG],
                                start=True, stop=True,
                            )
                            nc.scalar.activation(
                                out=f0rep[:, h * JG:(h + 1) * JG], in_=fb,
                                func=AF.Copy,
                            )
                    pg = p_pool.tile([P, grp, R], BF16, tag="pg")
                    pg_tiles.append(pg)
                    # X~ for 4 chunks, as 2 interleaved pairs ping-ponging
                    # the two PSUM scratch banks (LDWEIGHTS of one matmul
                    # overlaps the other bank's matmul stream)
                    for bb in (0, 2):
                        xa = ps1.tile([P, DEXT], F32, tag="xps", bufs=2)
                        xb = ps1.tile([P, DEXT], F32, tag="xps", bufs=2)
                        for kc in range(DK):
                            nc.tensor.matmul(
                                out=xa,
                                lhsT=nfc[:, kc, bb * P:(bb + 1) * P],
                                rhs=wextb[:, kc, :],
                                start=(kc == 0),
                                stop=(kc == DK - 1),
                            )
                            nc.tensor.matmul(
                                out=xb,
                                lhsT=nfc[:, kc, (bb + 1) * P:(bb + 2) * P],
                                rhs=wextb[:, kc, :],
                                start=(kc == 0),
                                stop=(kc == DK - 1),
                            )
                        # 258-wide casts: X -> 0:256, f0-junk -> 256 (ones
                        # re-memset below), f1 -> 257 (ACT bias reads it)
                        jc = g * grp + bb
                        nc.vector.tensor_copy(
                            out=xsb[:, jc, 0:DEXT], in_=xa[:, 0:DEXT]
                        )
                        nc.vector.tensor_copy(
                            out=xsb[:, jc + 1, 0:DEXT], in_=xb[:, 0:DEXT]
                        )
                    # restore the ones column for this group's 4 chunks
                    nc.vector.memset(xsb[:, g * grp:(g + 1) * grp, d_out], 1.0)

                for g in range(n_grp):
                    emit_group_x(g)
                    if g == 1:
                        emit_weights(0)
                        emit_weights(1)
                    elif g >= 2:
                        emit_weights(g)
                    if g >= 3:
                        if g == 3:
                            acc.append(sh1([P, DW], "acc5"))
                        emit_attn(g - 3, range(NIB1))
                emit_attn(n_grp - 3, range(NIB1))
                emit_attn(n_grp - 2, range(NIB1))
                emit_attn(n_grp - 1, range(NIB1))

                kexp = float(np.exp(-EXP_SHIFT))

                def emit_epilogue(ib):
                    # out_row = elu(num / rowsum); rowsum >= 1 always (every
                    # row has a neighbor and p >= 1), so no guard needed.
                    r = epi.tile([P, 1], F32, tag="r")
                    nc.vector.reciprocal(out=r, in_=acc[ib][:, d_out:DW])
                    u = epi.tile([P, d_out], F32, tag="u")
                    nc.vector.tensor_scalar(
                        out=u, in0=acc[ib][:, 0:d_out], scalar1=r, scalar2=None,
                        op0=ALU.mult,
                    )
                    rp = epi.tile([P, d_out], F32, tag="rp")
                    nc.vector.tensor_scalar(
                        out=rp, in0=u, scalar1=0.0, scalar2=-1.0,
                        op0=ALU.max, op1=ALU.add,
                    )
                    xm = epi.tile([P, d_out], F32, tag="xm")
                    nc.vector.tensor_scalar_min(xm, u, 0.0)
                    en = epi.tile([P, d_out], F32, tag="en")
                    nc.scalar.activation(out=en, in_=xm, func=AF.Exp, bias=b80)
                    res = epi.tile([P, d_out], F32, tag="res")
                    nc.vector.scalar_tensor_tensor(
                        out=res, in0=en, scalar=kexp, in1=rp,
                        op0=ALU.mult, op1=ALU.add,
                    )
                    nc.sync.dma_start(out=outp[ib * P:(ib + 1) * P, :], in_=res)

                # blocks 0-5 epilogue overlaps the 6-7 matmul tail
                for ib in range(NIB1):
                    emit_epilogue(ib)

                # tail: last two row-blocks through the freed X~ slots
                acc.append(ps1.tile([P, DEXT], F32, tag="xps", name="acc6",
                                    bufs=2)[:, 0:DW])
                acc.append(ps1.tile([P, DEXT], F32, tag="xps", name="acc7",
                                    bufs=2)[:, 0:DW])
                for chunk in range(NJ):
                    g, bb = divmod(chunk, grp)
                    for ib in (6, 7):
                        nc.tensor.matmul(
                            out=acc[ib],
                            lhsT=pg_tiles[g][:, bb, ib * P:(ib + 1) * P],
                            rhs=xsb[:, chunk, 0:DW],
                            start=(chunk == 0),
                            stop=(chunk == NJ - 1),
                        )
                emit_epilogue(6)
                emit_epilogue(7)

    _split_multi_waits(nc)
    return nc


_cached = {}

# Dev/test knobs (the grading harness just calls kernel(**inputs)):
_TRACE = False
_TMPDIR = None
_LAST_EXEC_NS = None
_LAST_RESULTS = None


def _get_program(n, d_in, d_out, m_cores):
    key = (n, d_in, d_out, m_cores)
    if key not in _cached:
        _cached[key] = build_gat(n, d_in, d_out, m_cores)
    return _cached[key]


def kernel(node_features, graph, W, v0, v1):
    import ml_dtypes

    node_features = np.asarray(node_features, dtype=np.float32)
    graph = np.asarray(graph)
    W = np.asarray(W, dtype=np.float32)
    v0 = np.asarray(v0, dtype=np.float32)
    v1 = np.asarray(v1, dtype=np.float32)

    n, d_in = node_features.shape
    d_out = W.shape[1]
    m = M_CORES
    R = n // m

    nc = _get_program(n, d_in, d_out, m)

    DK, n_grp, JG, grp = d_in // P, n // P // 4, 4 * P, 4
    wext = np.concatenate([W, W @ v0, W @ v1], axis=1).astype(ml_dtypes.bfloat16)
    wext_t = np.ascontiguousarray(
        wext.reshape(DK, P, d_out + 2).transpose(1, 0, 2).reshape(P, -1)
    )
    nfT = node_features.T.astype(ml_dtypes.bfloat16)
    mask01 = (graph != 0).astype(ml_dtypes.bfloat16)

    in_maps = []
    for c in range(m):
        rows = slice(c * R, (c + 1) * R)
        roll = -c * R
        nf_c = np.roll(nfT, roll, axis=1)          # [d_in, n], own cols first
        nf_t = np.ascontiguousarray(
            nf_c.reshape(DK, P, n_grp, JG).transpose(1, 2, 0, 3).reshape(P, -1)
        )
        m_c = np.roll(mask01[rows].T, roll, axis=0)    # [n, R]
        m_t = np.ascontiguousarray(
            m_c.reshape(n_grp, grp, P, R).transpose(2, 0, 1, 3).reshape(P, -1)
        )
        in_maps.append({"nfT": nf_t, "maskT": m_t, "wext": wext_t})
    global _LAST_EXEC_NS, _LAST_RESULTS
    res = run_bass_kernel_spmd(
        nc, in_maps, list(range(m)), trace=_TRACE, tmpdir=_TMPDIR
    )
    _LAST_EXEC_NS = res.exec_time_ns
    _LAST_RESULTS = res
    return np.concatenate([res.results[c]["out"] for c in range(m)], axis=0)


# revision 35
# speedup vs baseline: 1.1492x; 1.0134x over previous
"""GAT layer kernel for Trainium2, 8 NeuronCores.

Reference computation:
    X = node_features @ W            [N, DOUT]
    f0 = X @ v0 ; f1 = X @ v1       [N, 1]
    vals = sigmoid(f0 + f1.T) - 0.5
    alphas = softmax(where(graph != 0, vals, -inf), axis=1), masked to 0
    out = elu(alphas @ X)

Design notes:
  * softmax(sigmoid(z) - 0.5) == softmax(sigmoid(z)) (row-constant shift),
    so the softmax weights are w = exp(sigmoid(z)) up to a global scale.
  * w is evaluated in ONE activation pass through a patched ACT spline
    table: the `exp` buckets for |x| < 32 are refit to g(x)=exp(sigmoid(x))
    (bucket centers and ctrl tables unchanged, only cubic coefficients),
    while [32,128) keeps true exp so the elu epilogue can compute
    exp(u) = exp(u + 80) * e^-80 out of the un-patched region.
  * The adjacency mask ships as bf16 {0,1}, streamed by plain SWDGE DMA
    (tiled layout, 8KB contiguous runs per partition) on the otherwise-idle
    GpSimd queue, and applied multiplicatively with a single DVE
    tensor_tensor per group (bf16 2x packed mode). w >= 1 for unmasked
    entries so multiply-by-{0,1} is an exact mask.
  * softmax ratio: out_row = (sum_j p_ij X_j) / (sum_j p_ij); the row-sum
    comes free as a ones-column in the matmul rhs.
  * Two-phase schedule: ALL X~ chunks stream first (interleaved in pairs
    across the two PSUM scratch banks so LDWEIGHTS overlaps the previous
    matmul), freeing those banks before any attention matmul issues; the
    attention then runs 8 row-blocks wide with no tail sweep, overlapping
    the ACT weight pass which paces the second phase.
  * Row-sharding: each core owns N/8 output rows; each core recomputes
    X~ = nf @ [W | W@v0 | W@v1] for ALL rows from a replicated bf16 nf^T
    (collectives measured ~90us of barrier+skew on this fabric).
  * Per-core the j-axis is rotated host-side so the core's own rows come
    first: f0 (needed by every attention chunk) is computed from the
    first two streamed nf groups, with a K=1 matmul broadcasting the f0
    row to all 128 partitions. Softmax sums are order-invariant.
"""

import json
import os
import shutil
import tempfile

import numpy as np

# ----------------------------------------------------------------------------
# ACT table patch: refit exp's buckets to g(x)=exp(sigmoid(x)) for |x|<32,
# zero for x in (-128,-32], true exp kept for x in [32,128). Entry format
# (8 x fp32): [d0, d1, d2, d3, x0, 0, 0, 0], y = d0+d1 t+d2 t^2+d3 t^3,
# t = x - x0. Centers/ctrl/profile structure unchanged.
# ----------------------------------------------------------------------------

_ACT_SET = "exp_and_others"


def _g_target(x):
    return np.exp(1.0 / (1.0 + np.exp(-x)))


def _fit_cubic(f, lo, hi):
    x0 = 0.5 * (lo + hi)
    xs = x0 + 0.5 * (hi - lo) * np.cos(np.linspace(0, np.pi, 33))
    t = (xs - x0).astype(np.float64)
    A = np.stack([np.ones_like(t), t, t * t, t ** 3], axis=1)
    coef, *_ = np.linalg.lstsq(A, f(xs.astype(np.float64)), rcond=None)
    return coef, x0


def _patched_act_tables():
    """Write a patched copy of the pwp table dir; return act_info.json path."""
    from neuronxcc.driver.Job import Job

    src = os.path.join(Job.getPackageDir(), "pwp", "pwp_bin_trainium")
    dst = os.path.join(tempfile.gettempdir(), "gat_actpatch_v1")
    marker = os.path.join(dst, ".done")
    info = os.path.join(dst, "act_info.json")
    if os.path.exists(marker):
        return info
    shutil.rmtree(dst, ignore_errors=True)
    os.makedirs(dst)
    for f in os.listdir(src):
        shutil.copy(os.path.join(src, f), os.path.join(dst, f))
        os.chmod(os.path.join(dst, f), 0o644)

    bkt = np.fromfile(os.path.join(dst, f"{_ACT_SET}_bkt.bin"), dtype=np.float32)
    bkt = bkt.reshape(-1, 8).copy()
    prof = json.load(open(os.path.join(dst, f"{_ACT_SET}.json")))

    groups = {}
    for i in range(781):
        if i in (777, 778, 779, 780):
            continue
        x0 = float(bkt[i, 4])
        if x0 == 0.0:
            continue
        e = int(np.floor(np.log2(abs(x0))))
        groups.setdefault((np.sign(x0), e), []).append(i)
    for (sgn, e), idxs in groups.items():
        idxs.sort(key=lambda i: bkt[i, 4])
        centers = bkt[idxs, 4].astype(np.float64)
        w = float(np.min(np.diff(centers))) if len(idxs) > 1 else float(2.0 ** e)
        for i in idxs:
            x0 = float(bkt[i, 4])
            if x0 > 0 and e >= 5:
                continue          # keep true exp on [32, 128): elu epilogue
            if x0 < 0 and e >= 5:
                bkt[i, 0:4] = 0.0  # (-128, -32]: zero (unreachable margin)
                continue
            coef, _ = _fit_cubic(_g_target, x0 - w / 2, x0 + w / 2)
            bkt[i, 0:4] = coef.astype(np.float32)
    g0 = float(np.exp(0.5))
    for i in (777, 778):           # |x| < 2^-19 small-signal buckets
        bkt[i, 0:4] = [g0, 0.25 * g0, 0.5 * g0 * 0.25 ** 2, 0.0]
        bkt[i, 4] = 0.0
    for ent in prof["profile_meta_data"]:
        if ent["func_name"].startswith("exp_"):
            ent["fzero_result"] = int(np.float32(g0).view(np.uint32))

    bkt.astype(np.float32).tofile(os.path.join(dst, f"{_ACT_SET}_bkt.bin"))
    with open(os.path.join(dst, f"{_ACT_SET}.json"), "w") as fh:
        json.dump(prof, fh)
    open(marker, "w").close()
    return info


os.environ["BASS_ACT_ROOT_JSON_PATH"] = _patched_act_tables()

import concourse.bass as bass
import concourse.mybir as mybir
import concourse.tile as tile
from concourse.bass_utils import run_bass_kernel_spmd

# ----------------------------------------------------------------------------
# Workaround for "Too many sync wait commands": this walrus build accepts only
# ONE sync-wait per instruction. Post-pass: hoist surplus waits onto
# single-wait NOPs on the same engine, inserted immediately before the
# instruction (identical blocking semantics, per-engine order preserved).
# ----------------------------------------------------------------------------


def _split_multi_waits(nc):
    import bass_rust

    eng = {
        mybir.EngineType.PE: nc.tensor,
        mybir.EngineType.DVE: nc.vector,
        mybir.EngineType.Activation: nc.scalar,
        mybir.EngineType.Pool: nc.gpsimd,
        mybir.EngineType.SP: nc.sync,
    }
    for f in nc.m.functions:
        for blk in f.blocks:
            fixups = []  # (index, inst, waits)
            for idx, inst in enumerate(blk.instructions):
                si = inst.sync_info
                waits = list(si.on_wait) if si is not None and si.on_wait else []
                if len(waits) > 1 and inst.engine in eng:
                    fixups.append((idx, inst, waits))
            if not fixups:
                continue
            nops_by_idx = {}
            created = set()
            for idx, inst, waits in fixups:
                inst.sync_info.on_wait = [waits[-1]]
                nops = []
                for w in waits[:-1]:
                    nop = eng[inst.engine].nop(nofuse=True, hint="wait_split").ins
                    nop.sync_info = bass_rust.SyncInfo(on_wait=[w], on_update=[])
                    nops.append(nop)
                    created.add(id(nop))
                nops_by_idx[idx] = nops
            for b2 in f.blocks:
                b2.instructions[:] = [
                    i for i in b2.instructions if id(i) not in created
                ]
            new = []
            for idx, inst in enumerate(blk.instructions):
                new.extend(nops_by_idx.get(idx, ()))
                new.append(inst)
            blk.instructions[:] = new


# ----------------------------------------------------------------------------

F32 = mybir.dt.float32
BF16 = mybir.dt.bfloat16
AF = mybir.ActivationFunctionType
ALU = mybir.AluOpType

N, D_IN, D_OUT = 8192, 512, 256
M_CORES = 8
P = 128
EXP_SHIFT = 80.0  # elu exp computed as exp(u+80)*e^-80 (un-patched region)


def build_gat(n=N, d_in=D_IN, d_out=D_OUT, m_cores=M_CORES, grp=4, debug=False):
    """Per-core SPMD program. Inputs per core (j-axis rotated so own rows
    come first):
      nfT   [d_in, n]  bf16  -- node_features.T, columns rotated per core
      maskT [P, n_grp*grp*R] bf16 -- {1 unmasked, 0 masked}, tiled layout
      wext  [d_in, d_out+2] bf16 -- [W | W@v0 | W@v1]
    Output: out [R, d_out] f32 (this core's rows)."""
    R = n // m_cores
    NJ = n // P                  # 64 j-chunks
    IB = R // P                  # 8 output row-blocks
    DK = d_in // P               # 4 contraction chunks
    DEXT = d_out + 2
    DW = d_out + 1               # attention rhs: X | ones
    XW = d_out + 2               # xsb row: X | ones | f1
    n_grp = NJ // grp            # 16 groups
    JG = grp * P                 # 512 j per group

    nc = bass.Bass(num_devices=m_cores)
    # All inputs ship partition-major-tiled so each DMA has >=4KB contiguous
    # runs per partition (1KB-run APs measured ~75GB/s vs ~350 line rate):
    #   nfT[dp, g, kc, j]   = nf.T[kc*128+dp, g*512+j]
    #   maskT[dp, g, bb, i] = mask01.T[g*512+bb*128+dp, i]
    #   wext[dp, kc, c]     = [W|Wv0|Wv1][kc*128+dp, c]
    maskT = nc.declare_dram_parameter("maskT", [P, n_grp * grp * R], BF16, isOutput=False)
    nfT = nc.declare_dram_parameter("nfT", [P, n_grp * DK * JG], BF16, isOutput=False)
    wext = nc.declare_dram_parameter("wext", [P, DK * DEXT], BF16, isOutput=False)
    outp = nc.declare_dram_parameter("out", [R, d_out], F32, isOutput=True)

    with tile.TileContext(nc) as tc:
        with tc.tile_pool(name="persist", bufs=1) as persist, \
             tc.tile_pool(name="nfc", bufs=2) as nf_pool, \
             tc.tile_pool(name="mk", bufs=2) as mk_pool, \
             tc.tile_pool(name="pg", bufs=16) as p_pool, \
             tc.tile_pool(name="epi", bufs=2) as epi:

            wextb = persist.tile([P, DK, DEXT], BF16)
            nc.sync.dma_start(
                out=wextb,
                in_=bass.AP(wext, 0, [[DK * DEXT, P], [DEXT, DK], [1, DEXT]]),
            )
            # prewarm the (patched) exp table during the preamble
            warm = persist.tile([P, 1], F32)
            nc.gpsimd.memset(warm, 40.0)   # true-exp region, harmless
            nc.scalar.activation(out=warm, in_=warm, func=AF.Exp)

            # xsb row layout: [X (256) | ones (256) | f1 (257)]; the ones
            # column is re-memset per group after the 258-wide PSUM cast
            # (which drops f0-junk there); attention rhs reads cols 0:257.
            xsb = persist.tile([P, NJ, XW], BF16)
            f0rep = persist.tile([P, R], F32)
            # bf16 broadcast operands: one LDW+MM pair instead of the fp32
            # LOW/HIGH split (half the f0-chain latency); the row-constant
            # f0 rounding cancels in the softmax ratio
            f0flat = persist.tile([1, R], BF16)
            ones1 = persist.tile([1, P], BF16)
            nc.vector.memset(ones1, 1.0)
            b80 = persist.tile([P, 1], F32)
            nc.vector.memset(b80, EXP_SHIFT)

            pg_tiles = []
            mk_tiles = {}
            with tc.tile_pool(name="ps1", bufs=1, space="PSUM") as ps1:
                # 8 PSUM banks, fully subscribed:
                #   5 dedicated attention accumulators (row-blocks 0-4)
                #   1 shared bank, serially: f0 row psums -> f0 broadcast
                #     halves -> 6th accumulator (row-block 5)
                #   2 X~ tile slots; after the last X~ tile their slots
                #     become accumulators for row-blocks 6-7 (tail).
                # Row-blocks 0-5 accumulate DURING the X~/ACT streaming
                # phase (interleaved per group, 2 groups behind so PE never
                # waits on the mask+mult latency); 6-7 run as a short tail.
                acc = [
                    ps1.tile([P, DW], F32, tag=f"acc{ib}", name=f"acc{ib}")
                    for ib in range(5)
                ]
                NIB1 = 6

                def sh1(shape, name):
                    return ps1.tile(shape, F32, tag="sh1", name=name, bufs=1)

                def emit_attn(g, ibs):
                    for bb in range(grp):
                        chunk = g * grp + bb
                        for ib in ibs:
                            nc.tensor.matmul(
                                out=acc[ib],
                                lhsT=pg_tiles[g][:, bb, ib * P:(ib + 1) * P],
                                rhs=xsb[:, chunk, 0:DW],
                                start=(chunk == 0),
                                stop=(chunk == NJ - 1),
                            )

                def emit_weights(g):
                    """ACT pass + multiplicative {0,1} mask for group g. Must
                    only be emitted once f0rep's write has been emitted (reads
                    emitted before a tile's first write see garbage)."""
                    # mask DMAs are issued here, one group ahead of use, so
                    # they trail the ramp-critical nf0/nf1 transfers (first
                    # matmul measured waiting to 15.6us on nf0 when masks
                    # were issued eagerly) but still land a full weights
                    # cycle before their TT consumes them
                    if g == 0:
                        issue_mask(0)
                    if g + 1 < n_grp:
                        issue_mask(g + 1)
                    pg = pg_tiles[g]
                    # one ACT pass: p = g(f0_i + f1_j) = exp(sigmoid(z))
                    for bb in range(grp):
                        jc = g * grp + bb
                        nc.scalar.activation(
                            out=pg[:, bb, :],
                            in_=f0rep,
                            func=AF.Exp,
                            bias=xsb[:, jc, d_out + 1:d_out + 2],
                            scale=1.0,
                        )
                    # mask: one fused bf16 multiply (2x packed DVE mode);
                    # p >= 1 for unmasked entries so {0,1}-multiply is exact
                    mk = mk_tiles.pop(g)
                    nc.vector.tensor_tensor(
                        out=pg.rearrange("p g r -> p (g r)"),
                        in0=pg.rearrange("p g r -> p (g r)"),
                        in1=mk.rearrange("p g r -> p (g r)"),
                        op=ALU.mult,
                    )

                f0ps = [None, None]

                def issue_mask(gm):
                    # SWDGE on the idle GpSimd queue so its buffer-waits
                    # never stall the nf stream
                    mk = mk_pool.tile([P, grp, R], BF16, tag="mk")
                    mk_tiles[gm] = mk
                    nc.gpsimd.dma_start(
                        out=mk,
                        in_=bass.AP(
                            maskT, gm * grp * R,
                            [[n_grp * grp * R, P], [R, grp], [1, R]],
                        ),
                    )

                def emit_group_x(g):
                    """nf DMA, f0 rows (g<2), X~ matmuls + casts."""
                    nfc = nf_pool.tile([P, DK, JG], BF16, tag="nfc")
                    nc.sync.dma_start(
                        out=nfc,
                        in_=bass.AP(
                            nfT, g * DK * JG,
                            [[n_grp * DK * JG, P], [JG, DK], [1, JG]],
                        ),
                    )

                    # f0 row for own rows (groups 0-1 under rotation), via
                    # stationary wv0 column (M=1) -> row-form, no transpose
                    if g < 2:
                        f0ps[g] = sh1([1, JG], f"f0ps{g}")
                        for kc in range(DK):
                            nc.tensor.matmul(
                                out=f0ps[g],
                                lhsT=wextb[:, kc, d_out:d_out + 1],
                                rhs=nfc[:, kc, :],
                                start=(kc == 0),
                                stop=(kc == DK - 1),
                            )
                        # ScE copy: keeps the f0 chain off the busy DVE
                        nc.scalar.activation(
                            out=f0flat[:, g * JG:(g + 1) * JG], in_=f0ps[g],
                            func=AF.Copy,
                        )
                    if g == 1:
                        # broadcast f0 BEFORE X~(1) in the PE stream: the
                        # first ACT then waits only on group 0's casts, not
                        # on 32 cold X~ matmuls queued ahead of fb
                        for h in range(R // JG):
                            fb = sh1([P, JG], f"f0bc{h}")
                            nc.tensor.matmul(
                                out=fb,
                                lhsT=ones1,
                                rhs=f0flat[:, h * JG:(h + 1) * JG],
                                start=True, stop=True,
                            )
                            nc.scalar.activation(
                                out=f0rep[:, h * JG:(h + 1) * JG], in_=fb,
                                func=AF.Copy,
                            )
                    pg = p_pool.tile([P, grp, R], BF16, tag="pg")
                    pg_tiles.append(pg)
                    # X~ for 4 chunks, as 2 interleaved pairs ping-ponging
                    # the two PSUM scratch banks (LDWEIGHTS of one matmul
                    # overlaps the other bank's matmul stream)
                    for bb in (0, 2):
                        xa = ps1.tile([P, DEXT], F32, tag="xps", bufs=2)
                        xb = ps1.tile([P, DEXT], F32, tag="xps", bufs=2)
                        for kc in range(DK):
                            nc.tensor.matmul(
                                out=xa,
                                lhsT=nfc[:, kc, bb * P:(bb + 1) * P],
                                rhs=wextb[:, kc, :],
                                start=(kc == 0),
                                stop=(kc == DK - 1),
                            )
                            nc.tensor.matmul(
                                out=xb,
                                lhsT=nfc[:, kc, (bb + 1) * P:(bb + 2) * P],
                                rhs=wextb[:, kc, :],
                                start=(kc == 0),
                                stop=(kc == DK - 1),
                            )
                        # 258-wide casts: X -> 0:256, f0-junk -> 256 (ones
                        # re-memset below), f1 -> 257 (ACT bias reads it)
                        jc = g * grp + bb
                        nc.vector.tensor_copy(
                            out=xsb[:, jc, 0:DEXT], in_=xa[:, 0:DEXT]
                        )
                        nc.vector.tensor_copy(
                            out=xsb[:, jc + 1, 0:DEXT], in_=xb[:, 0:DEXT]
                        )
                    # restore the ones column for this group's 4 chunks
                    nc.vector.memset(xsb[:, g * grp:(g + 1) * grp, d_out], 1.0)

                for g in range(n_grp):
                    emit_group_x(g)
                    if g == 1:
                        emit_weights(0)
                        emit_weights(1)
                    elif g >= 2:
                        emit_weights(g)
                    if g >= 3:
                        if g == 3:
                            acc.append(sh1([P, DW], "acc5"))
                        emit_attn(g - 3, range(NIB1))
                emit_attn(n_grp - 3, range(NIB1))
                emit_attn(n_grp - 2, range(NIB1))
                emit_attn(n_grp - 1, range(NIB1))

                kexp = float(np.exp(-EXP_SHIFT))

                def emit_epilogue(ib):
                    # out_row = elu(num / rowsum); rowsum >= 1 always (every
                    # row has a neighbor and p >= 1), so no guard needed.
                    r = epi.tile([P, 1], F32, tag="r")
                    nc.vector.reciprocal(out=r, in_=acc[ib][:, d_out:DW])
                    u = epi.tile([P, d_out], F32, tag="u")
                    nc.vector.tensor_scalar(
                        out=u, in0=acc[ib][:, 0:d_out], scalar1=r, scalar2=None,
                        op0=ALU.mult,
                    )
                    rp = epi.tile([P, d_out], F32, tag="rp")
                    nc.vector.tensor_scalar(
                        out=rp, in0=u, scalar1=0.0, scalar2=-1.0,
                        op0=ALU.max, op1=ALU.add,
                    )
                    xm = epi.tile([P, d_out], F32, tag="xm")
                    nc.vector.tensor_scalar_min(xm, u, 0.0)
                    en = epi.tile([P, d_out], F32, tag="en")
                    nc.scalar.activation(out=en, in_=xm, func=AF.Exp, bias=b80)
                    res = epi.tile([P, d_out], F32, tag="res")
                    nc.vector.scalar_tensor_tensor(
                        out=res, in0=en, scalar=kexp, in1=rp,
                        op0=ALU.mult, op1=ALU.add,
                    )
                    nc.sync.dma_start(out=outp[ib * P:(ib + 1) * P, :], in_=res)

                # blocks 0-5 epilogue overlaps the 6-7 matmul tail
                for ib in range(NIB1):
                    emit_epilogue(ib)

                # tail: last two row-blocks through the freed X~ slots
                acc.append(ps1.tile([P, DEXT], F32, tag="xps", name="acc6",
                                    bufs=2)[:, 0:DW])
                acc.append(ps1.tile([P, DEXT], F32, tag="xps", name="acc7",
                                    bufs=2)[:, 0:DW])
                for chunk in range(NJ):
                    g, bb = divmod(chunk, grp)
                    for ib in (6, 7):
                        nc.tensor.matmul(
                            out=acc[ib],
                            lhsT=pg_tiles[g][:, bb, ib * P:(ib + 1) * P],
                            rhs=xsb[:, chunk, 0:DW],
                            start=(chunk == 0),
                            stop=(chunk == NJ - 1),
                        )
                emit_epilogue(6)
                emit_epilogue(7)

    _split_multi_waits(nc)
    return nc


_cached = {}

# Dev/test knobs (the grading harness just calls kernel(**inputs)):
_TRACE = False
_TMPDIR = None
_LAST_EXEC_NS = None
_LAST_RESULTS = None


def _get_program(n, d_in, d_out, m_cores):
    key = (n, d_in, d_out, m_cores)
    if key not in _cached:
        _cached[key] = build_gat(n, d_in, d_out, m_cores)
    return _cached[key]


def kernel(node_features, graph, W, v0, v1):
    import ml_dtypes

    node_features = np.asarray(node_features, dtype=np.float32)
    graph = np.asarray(graph)
    W = np.asarray(W, dtype=np.float32)
    v0 = np.asarray(v0, dtype=np.float32)
    v1 = np.asarray(v1, dtype=np.float32)

    n, d_in = node_features.shape
    d_out = W.shape[1]
    m = M_CORES
    R = n // m

    nc = _get_program(n, d_in, d_out, m)

    DK, n_grp, JG, grp = d_in // P, n // P // 4, 4 * P, 4
    wext = np.concatenate([W, W @ v0, W @ v1], axis=1).astype(ml_dtypes.bfloat16)
    wext_t = np.ascontiguousarray(
        wext.reshape(DK, P, d_out + 2).transpose(1, 0, 2).reshape(P, -1)
    )
    nfT = node_features.T.astype(ml_dtypes.bfloat16)
    mask01 = (graph != 0).astype(ml_dtypes.bfloat16)

    in_maps = []
    for c in range(m):
        rows = slice(c * R, (c + 1) * R)
        roll = -c * R
        nf_c = np.roll(nfT, roll, axis=1)          # [d_in, n], own cols first
        nf_t = np.ascontiguousarray(
            nf_c.reshape(DK, P, n_grp, JG).transpose(1, 2, 0, 3).reshape(P, -1)
        )
        m_c = np.roll(mask01[rows].T, roll, axis=0)    # [n, R]
        m_t = np.ascontiguousarray(
            m_c.reshape(n_grp, grp, P, R).transpose(2, 0, 1, 3).reshape(P, -1)
        )
        in_maps.append({"nfT": nf_t, "maskT": m_t, "wext": wext_t})
    global _LAST_EXEC_NS, _LAST_RESULTS
    res = run_bass_kernel_spmd(
        nc, in_maps, list(range(m)), trace=_TRACE, tmpdir=_TMPDIR
    )
    _LAST_EXEC_NS = res.exec_time_ns
    _LAST_RESULTS = res
    return np.concatenate([res.results[c]["out"] for c in range(m)], axis=0)
